# revision 55
# baseline (speedup 1.0000x reference)
"""Trainium2 Bass kernel for nn_AttentionFlow (BiDAF-style attention flow).

Math (per batch b, all biases cancel):
  s[t,i]   = <c_t,w_c> + <q_i,w_q> + <c_t*q_i, w_cq>  (+ biases)
  a        = softmax_i(s)          -> c2q = a @ q
  beta     = softmax_t(max_i s)    -> q2c = beta^T c
  out      = [c | c2q | c*c2q | c*q2c]

Key identities:
  * softmax_i(s[t,:]) is invariant to the per-row term sc[t] and all biases.
  * sc is folded into the matmul weights: qa[d,i] = q^T[d,i]*w_cq[d]+w_c[d].
  * exp(max_i s) = max_i exp(s), so beta's numerator comes from a max over
    the already-exponentiated E with no extra exp.
  * t and i orderings are arbitrary (softmax/sums are order-invariant and
    outputs are re-addressed by AP); i is stored as i = 4p + k.

Shipped kernel (_build_v3, KERNEL_OPTS): computes s TRANSPOSED per pair of
128-row tiles (256 t columns, which keeps f32r matmuls at full rate):
  s^T[i,t] = sum_d qa[d,i] C^T[d,t];  E^T = exp(s^T + sq[i]) via the Act
engine with sq as a per-partition bias (free); c2q and the row sums r come
from mm2 with E^T chunks stationary against q rows in natural layout plus a
ones column — no E transpose and no sq matmul. beta's g = max_i E^T via a
bf16 max tree + one 128-wide PE transpose. c/q live in SBUF as f32r
(DMA-bitcast) so every matmul and PE transpose runs at f32r rate; the
f32->f32r "rounded producer" rule is satisfied because DMA and DVE outputs
count as rounded (Act does NOT - it faults on HW - and tensor_tensor_reduce
faults outright).

Perf model (per core): 21MB HBM traffic (c 4MB in, q 1MB in, out 16MB) at
~360GB/s/core means a ~58us DMA roofline; engine busy (cost model) is
PE 40us, DVE 40us, SP 33us, Act 31us, Pool 26us - all below the roofline,
so the kernel is DMA-bound when the shared terminal is quiet and degrades
proportionally to HBM contention. Output descriptors are 2KB+/partition,
o1 (=c) goes out as one 4MB DMA, out-DMA dispatch alternates SP/Pool rings
and input dispatch alternates SP/Act rings so no single sequencer
serializes the stream.

Sharding: data-parallel over batch, one batch element per NeuronCore (8).
"""

import numpy as np

N_CORES = 8
T, I, D = 2048, 512, 512
TT = T // 128  # 16 row tiles
KC = 4         # 128-chunks of D (and of I)

DEFAULT_OPTS = dict(contig_in=True, out_ring="sync", two_pass=False,
                    skip_out=False, memset_in=False, dma_pair=False,
                    bufs_work=3, bufs_out=3, out_split=False,
                    dup_pe=False, dup_dve=False, dup_act=False, bloat=0,
                    act_copies="act", exp_accum=True, g_accum=True,
                    mul_eng="gpsimd", batch_recip=False, alt_copies=False,
                    split_in=True, q2c_inline=False, o4_split=True,
                    ps_tr_bufs=2, ct_eng="dve", early_cout=False,
                    ps_s_bufs=2, ps_mm2_bufs=2, fine_tiles=True, fine_c=False,
                    c_onebuf=True, o1_batch=True, tr_f32r=False,
                    q2c_f32r=False, out_alt=True, c_f32r=True)

_BUILT = None


def _build_v3(reps=1, timing_mode=False, opts=None):
    """s-transposed formulation.

    Per pair of 128-row tiles (256 t-columns, satisfying the f32r
    ap>=256 full-rate rule):
      s^T[i,t] = sum_d qa[d,i] * C^T[d,t]   (qa = Q^T*wcq + wc, folds sc)
      E^T = exp(s^T + sq[i])                (sq per-partition Act bias)
      g[t] = max_i E^T[i,t]  (= exp(max_i s) by monotonicity; bf16 max
             tree + one 128-wide PE transpose + free-axis reduce_max)
      c2q-row r[t] and c2q via mm2 with E^T chunks as stationary weights
             (rhs = q rows natural layout; ones column gives r).
    No E transpose, no sq matmul, q2c contraction in f32r.
    """
    import concourse.tile as tile
    from concourse import bacc, mybir
    from concourse.masks import make_identity

    o = dict(v3_mul_split=True, out_alt=True, o1_batch=True,
             seq_pr=False, sq_mm=False, in_alt=False, bufs_work=3,
             bufs_out=3, ct_alt=False)
    if opts:
        o.update(opts)

    f32 = mybir.dt.float32
    f32r = mybir.dt.float32r
    bf16 = mybir.dt.bfloat16
    AF = mybir.ActivationFunctionType
    AX = mybir.AxisListType
    ALU = mybir.AluOpType
    IC = 4

    nc = bacc.Bacc("TRN2", target_bir_lowering=False, debug=False,
                   num_devices=N_CORES)
    c_d = nc.dram_tensor("c", [T, D], f32, kind="ExternalInput").ap()
    q_d = nc.dram_tensor("q", [I, D], f32, kind="ExternalInput").ap()
    wc_d = nc.dram_tensor("wc", [D], f32, kind="ExternalInput").ap()
    wq_d = nc.dram_tensor("wq", [D], f32, kind="ExternalInput").ap()
    wcq_d = nc.dram_tensor("wcq", [D], f32, kind="ExternalInput").ap()
    out_kind = "Internal" if timing_mode else "ExternalOutput"
    out_d = nc.dram_tensor("out", [T, 4 * D], f32, kind=out_kind).ap()
    tick_d = (nc.dram_tensor("tick", [1, 1], f32, kind="ExternalOutput").ap()
              if timing_mode else None)

    with tile.TileContext(nc) as tc:
        with (
            tc.tile_pool(name="const", bufs=1) as constp,
            tc.tile_pool(name="big", bufs=1) as bigp,
            tc.tile_pool(name="work", bufs=o["bufs_work"]) as workp,
            tc.tile_pool(name="outp", bufs=o["bufs_out"]) as outp,
            tc.tile_pool(name="ps_tr", bufs=2, space="PSUM") as ps_tr,
            tc.tile_pool(name="ps_s", bufs=3, space="PSUM") as ps_s,
            tc.tile_pool(name="ps_mm2", bufs=2, space="PSUM") as ps_mm2,
            tc.tile_pool(name="ps_sm", bufs=1, space="PSUM") as ps_sm,
        ):
            for _rep in range(reps):
                # ---------------- setup --------------------------------------
                ident_f = constp.tile([128, 128], f32, tag="idf")
                make_identity(nc, ident_f[:])
                ident_b = constp.tile([128, 128], bf16, tag="idb")
                make_identity(nc, ident_b[:])
                ident_r = constp.tile([128, 128], f32r, tag="idr")
                nc.vector.tensor_copy(ident_r[:], ident_f[:])
                ones_row_f = constp.tile([1, 128], f32, tag="ones_row_f")
                nc.vector.memset(ones_row_f[:], 1.0)
                ones_row = constp.tile([1, 128], f32r, tag="ones_row")
                nc.vector.tensor_copy(ones_row[:], ones_row_f[:])
                ones_col = constp.tile([128, 1], f32, tag="ones_col")
                nc.vector.memset(ones_col[:], 1.0)
                ones_col_b = constp.tile([128, 1], bf16, tag="ones_col_b")
                nc.vector.memset(ones_col_b[:], 1.0)

                wcq_col = constp.tile([128, KC], f32, tag="wcq_col")
                nc.sync.dma_start(wcq_col[:],
                                  wcq_d.rearrange("(a b) -> b a", b=128))
                wc_col = constp.tile([128, KC], f32, tag="wc_col")
                nc.sync.dma_start(wc_col[:],
                                  wc_d.rearrange("(a b) -> b a", b=128))
                wq_row = constp.tile([1, D], f32, tag="wq_row")
                nc.sync.dma_start(wq_row[:],
                                  wq_d.rearrange("(a d) -> a d", a=1))

                # q in [i_part, d] with i = 4p+k; f32r so PE transposes and
                # setup matmuls run at full f32r rate
                q_sb = bigp.tile([128, KC, D], f32r, tag="q_sb")
                nc.sync.dma_start(
                    q_sb[:],
                    q_d.rearrange("(p k) d -> p k d", k=KC).bitcast(f32r))
                q_bf = bigp.tile([128, KC, D], bf16, tag="q_bf")
                nc.vector.tensor_copy(q_bf[:], q_sb[:].bitcast(f32))

                # c in f32r, one contiguous buffer, fine-grained loads
                crs = c_d.rearrange("(p j) d -> p j d", j=TT)
                c_big = bigp.tile([128, TT, D], f32r, tag="c_big")
                for _j in range(TT):
                    in_eng = (nc.scalar if (o["in_alt"] and _j % 2)
                              else nc.sync)
                    in_eng.dma_start(c_big[:, _j, :],
                                     crs[:, _j, :].bitcast(f32r))

                ors = out_d.rearrange("(p j) w -> p j w", j=TT)

                _out_n = [0]

                def out_dma(j, sl, src):
                    _out_n[0] += 1
                    eng = (nc.gpsimd if (o["out_alt"] and _out_n[0] % 2)
                           else nc.sync)
                    eng.dma_start(ors[:, j, sl], src)

                # o1 = c passthrough, one big DMA
                if o["o1_batch"]:
                    nc.sync.dma_start(ors[:, :, 0:512], c_big[:].bitcast(f32))

                sq_col = constp.tile([128, KC], f32, tag="sq_col")
                if not o["sq_mm"]:
                    # wq broadcast to all partitions (via PE), then per-row
                    # dots: sq_col[p, k] = <q[4p+k, :], wq>
                    ps_bc = ps_mm2.tile([128, D], f32, tag="pc")
                    nc.tensor.matmul(ps_bc[:], ones_row_f[:], wq_row[:],
                                     start=True, stop=True)
                    wq_bc = constp.tile([128, D], f32, tag="wq_bc")
                    nc.scalar.copy(wq_bc[:], ps_bc[:])
                    sq_scr = constp.tile([128, D], f32, tag="sq_scr")
                    for k in range(KC):
                        nc.vector.tensor_tensor_reduce(
                            sq_scr[:], q_sb[:, k], wq_bc[:], 1.0, 0.0,
                            ALU.mult, ALU.add, accum_out=sq_col[:, k:k + 1])

                # qa[d, i] = Q^T * wcq + wc  (via PE transposes of q)
                qa = bigp.tile([128, KC, I], f32r, tag="qa")
                if o["sq_mm"]:
                    qt = bigp.tile([128, KC, I], f32r, tag="qt")
                else:
                    qt = None
                for k in range(KC):
                    pt = ps_mm2.tile([128, I], f32, tag="pc")
                    for ik in range(KC):
                        nc.tensor.transpose(
                            pt[:, ik * 128:(ik + 1) * 128].bitcast(f32r),
                            q_sb[:, ik, k * 128:(k + 1) * 128],
                            ident_r[:])
                    if o["sq_mm"]:
                        nc.vector.tensor_copy(qt[:, k], pt[:])
                    nc.vector.tensor_scalar(
                        qa[:, k], pt[:], wcq_col[:, k:k + 1],
                        wc_col[:, k:k + 1], op0=ALU.mult, op1=ALU.add)

                if o["sq_mm"]:
                    # sq_row = wq^T Q^T, then 4 thin transposes into sq_col
                    wq_col = constp.tile([128, KC], f32r, tag="wq_col")
                    nc.sync.dma_start(
                        wq_col[:],
                        wq_d.rearrange("(a b) -> b a", b=128).bitcast(f32r))
                    ps_sq = ps_mm2.tile([1, I], f32, tag="pc")
                    for k in range(KC):
                        nc.tensor.matmul(ps_sq[:], wq_col[:, k:k + 1],
                                         qt[:, k], start=(k == 0),
                                         stop=(k == KC - 1))
                    sq_row = constp.tile([1, I], f32, tag="sq_row")
                    nc.scalar.copy(sq_row[:], ps_sq[:])
                    sq_ps = ps_sm.tile([128, KC], f32, tag="pr")
                    for ic in range(IC):
                        nc.tensor.transpose(
                            sq_ps[:, ic:ic + 1],
                            sq_row[0:1, ic * 128:(ic + 1) * 128],
                            ones_row_f[0:1, 0:1])
                    nc.vector.tensor_copy(sq_col[:], sq_ps[:])

                g = constp.tile([128, TT], f32r, tag="g")
                ri_tiles = []
                for _j in range(TT):
                    ri_j = bigp.tile([128, 1], f32, tag=f"ri{_j}")
                    ri_tiles.append(ri_j)

                # ---------------- phase 1: per pair of tiles -----------------
                for m in range(TT // 2):
                    j0 = 2 * m

                    # C^T for the pair: [d_part, 256 t]
                    ct = workp.tile([128, KC, 256], f32r, tag="ct")
                    for k in range(KC):
                        ptk = ps_tr.tile([128, 256], f32, tag="ptk")
                        for jj in range(2):
                            nc.tensor.transpose(
                                ptk[:, jj * 128:(jj + 1) * 128]
                                .bitcast(f32r),
                                c_big[:, j0 + jj,
                                      k * 128:(k + 1) * 128],
                                ident_r[:])
                        if o["ct_alt"] and k % 2:
                            nc.gpsimd.tensor_copy(ct[:, k], ptk[:])
                        else:
                            nc.vector.tensor_copy(ct[:, k], ptk[:])

                    # mm1 per i-chunk + exp + bf16 max tree
                    et2 = workp.tile([128, IC, 256], bf16, tag="et2")
                    m4e = workp.tile([128, 256], bf16, tag="m4e")
                    for ic in range(IC):
                        psT = ps_s.tile([128, 256], f32, tag="psT")
                        for k in range(KC):
                            nc.tensor.matmul(
                                psT[:], qa[:, k, ic * 128:(ic + 1) * 128],
                                ct[:, k], start=(k == 0), stop=(k == KC - 1))
                        nc.scalar.activation(et2[:, ic, :], psT[:], AF.Exp,
                                             bias=sq_col[:, ic:ic + 1])
                        if ic == 0:
                            nc.vector.tensor_copy(m4e[:], et2[:, 0, :])
                        else:
                            nc.vector.tensor_tensor(
                                m4e[:], m4e[:], et2[:, ic, :], ALU.max)

                    # per tile: g column, mm2, epilogue
                    for jj in range(2):
                        j = j0 + jj
                        mt = ps_tr.tile([128, 128], bf16, tag="ptk")
                        nc.tensor.transpose(
                            mt[:], m4e[:, jj * 128:(jj + 1) * 128],
                            ident_b[:])
                        nc.vector.reduce_max(g[:, j:j + 1], mt[:], axis=AX.X)

                        pc = ps_mm2.tile([128, 512], f32, tag="pc")
                        pr = ps_sm.tile([128, 1], f32, tag="pr")
                        if o["seq_pr"]:
                            for ic in range(IC):
                                nc.tensor.matmul(
                                    pc[:], et2[:, ic, jj * 128:(jj + 1) * 128],
                                    q_bf[:, ic], start=(ic == 0),
                                    stop=(ic == IC - 1))
                            for ic in range(IC):
                                nc.tensor.matmul(
                                    pr[:], et2[:, ic, jj * 128:(jj + 1) * 128],
                                    ones_col_b[:], start=(ic == 0),
                                    stop=(ic == IC - 1))
                        else:
                            for ic in range(IC):
                                lhs = et2[:, ic, jj * 128:(jj + 1) * 128]
                                nc.tensor.matmul(pc[:], lhs, q_bf[:, ic],
                                                 start=(ic == 0),
                                                 stop=(ic == IC - 1))
                                nc.tensor.matmul(pr[:], lhs, ones_col_b[:],
                                                 start=(ic == 0),
                                                 stop=(ic == IC - 1),
                                                 skip_group_check=True)
                        nc.vector.reciprocal(ri_tiles[j][:], pr[:])
                        o_t = outp.tile([128, 1024], f32, tag="o23")
                        nc.scalar.mul(o_t[:, 0:512], pc[:], ri_tiles[j][:])
                        mul_e = (nc.gpsimd if (o["v3_mul_split"] and j % 2)
                                 else nc.vector)
                        mul_e.tensor_mul(o_t[:, 512:1024],
                                         c_big[:, j].bitcast(f32),
                                         o_t[:, 0:512])
                        out_dma(j, slice(512, 1536), o_t[:])

                # ---------------- phase 2: q2c -------------------------------
                gsum = constp.tile([128, 1], f32, tag="gsum")
                nc.vector.reduce_sum(gsum[:], g[:], axis=AX.X)
                psZ = ps_sm.tile([1, 1], f32, tag="pr")
                nc.tensor.matmul(psZ[:], ones_col[:], gsum[:],
                                 start=True, stop=True)
                psq2c = ps_mm2.tile([1, D], f32, tag="pc")
                for j in range(TT):
                    nc.tensor.matmul(psq2c[:], g[:, j:j + 1], c_big[:, j],
                                     start=(j == 0), stop=(j == TT - 1))
                Zinv = constp.tile([1, 1], f32, tag="Zinv")
                nc.vector.reciprocal(Zinv[:], psZ[:])
                q2c_row = constp.tile([1, D], f32r, tag="q2c_row")
                nc.vector.tensor_scalar_mul(q2c_row[:], psq2c[:], Zinv[:])
                psbc = ps_mm2.tile([128, D], f32, tag="pc")
                nc.tensor.matmul(psbc[:], ones_row[:], q2c_row[:],
                                 start=True, stop=True)
                q2c_bc = constp.tile([128, D], f32, tag="q2c_bc")
                nc.scalar.copy(q2c_bc[:], psbc[:])

                # ---------------- phase 3: o4 --------------------------------
                for j in range(TT):
                    mul_e4 = nc.gpsimd if j % 2 else nc.vector
                    o4 = outp.tile([128, D], f32, tag="o4")
                    mul_e4.tensor_mul(o4[:], c_big[:, j].bitcast(f32),
                                      q2c_bc[:])
                    out_dma(j, slice(1536, 2048), o4[:])

        if timing_mode:
            with tc.tile_pool(name="tickp", bufs=1) as tickp:
                tk = tickp.tile([1, 1], f32, tag="tick")
                nc.vector.memset(tk[:], 1.0)
                nc.sync.dma_start(tick_d[:], tk[:])

    nc.compile()
    return nc


def _build(reps=1, timing_mode=False, opts=None):
    if opts and opts.get("v3"):
        o2 = {k: v for k, v in opts.items() if k != "v3"}
        return _build_v3(reps, timing_mode, o2)
    import concourse.tile as tile
    from concourse import bacc, mybir
    from concourse.masks import make_identity

    o = dict(DEFAULT_OPTS)
    if opts:
        o.update(opts)

    f32 = mybir.dt.float32
    f32r = mybir.dt.float32r
    bf16 = mybir.dt.bfloat16
    AF = mybir.ActivationFunctionType
    AX = mybir.AxisListType
    ALU = mybir.AluOpType

    nc = bacc.Bacc("TRN2", target_bir_lowering=False, debug=False,
                   num_devices=N_CORES)
    c_d = nc.dram_tensor("c", [T, D], f32, kind="ExternalInput").ap()
    q_d = nc.dram_tensor("q", [I, D], f32, kind="ExternalInput").ap()
    wc_d = nc.dram_tensor("wc", [D], f32, kind="ExternalInput").ap()
    wq_d = nc.dram_tensor("wq", [D], f32, kind="ExternalInput").ap()
    wcq_d = nc.dram_tensor("wcq", [D], f32, kind="ExternalInput").ap()
    out_kind = "Internal" if timing_mode else "ExternalOutput"
    out_d = nc.dram_tensor("out", [T, 4 * D], f32, kind=out_kind).ap()
    tick_d = (nc.dram_tensor("tick", [1, 1], f32, kind="ExternalOutput").ap()
              if timing_mode else None)

    out_eng = {"sync": nc.sync, "scalar": nc.scalar, "gpsimd": nc.gpsimd,
               "vector": nc.vector}[o["out_ring"]]

    with tile.TileContext(nc) as tc:
        with (
            tc.tile_pool(name="const", bufs=1) as constp,
            tc.tile_pool(name="big", bufs=1) as bigp,
            tc.tile_pool(name="work", bufs=o["bufs_work"]) as workp,
            tc.tile_pool(name="outp", bufs=o["bufs_out"]) as outp,
            tc.tile_pool(name="ps_tr", bufs=o["ps_tr_bufs"],
                         space="PSUM") as ps_tr,
            tc.tile_pool(name="ps_acc", bufs=1, space="PSUM") as ps_acc,
            tc.tile_pool(name="ps_s", bufs=o["ps_s_bufs"],
                         space="PSUM") as ps_s,
            tc.tile_pool(name="ps_mm2", bufs=o["ps_mm2_bufs"],
                         space="PSUM") as ps_mm2,
        ):
            for _rep in range(reps):
                # ---------------- phase 0 -----------------------------------
                ident_f = constp.tile([128, 128], f32, tag="idf")
                make_identity(nc, ident_f[:])
                ident_b = constp.tile([128, 128], bf16, tag="idb")
                make_identity(nc, ident_b[:])

                if o["c_f32r"]:
                    ident_rt = constp.tile([128, 128], f32r, tag="idr")
                    nc.vector.tensor_copy(ident_rt[:], ident_f[:])
                    ident_r = ident_rt[:]
                ones_row_f = constp.tile([1, 128], f32, tag="ones_row_f")
                nc.vector.memset(ones_row_f[:], 1.0)
                ones_row = constp.tile([1, 128], f32r, tag="ones_row")
                nc.vector.tensor_copy(ones_row[:], ones_row_f[:])
                ones_col = constp.tile([128, 1], f32, tag="ones_col")
                nc.vector.memset(ones_col[:], 1.0)

                wcq_col = constp.tile([128, KC], f32, tag="wcq_col")
                nc.sync.dma_start(wcq_col[:],
                                  wcq_d.rearrange("(a b) -> b a", b=128))
                wc_col = constp.tile([128, KC], f32, tag="wc_col")
                nc.sync.dma_start(wc_col[:],
                                  wc_d.rearrange("(a b) -> b a", b=128))
                wq_col = constp.tile([128, KC], f32, tag="wq_col")
                nc.sync.dma_start(wq_col[:],
                                  wq_d.rearrange("(a b) -> b a", b=128))

                q_sb = bigp.tile([128, KC, D], f32, tag="q_sb")
                if o["memset_in"]:
                    nc.gpsimd.memset(q_sb[:], 0.01)
                elif o["contig_in"]:
                    nc.sync.dma_start(
                        q_sb[:], q_d.rearrange("(p k) d -> p k d", k=KC))
                else:
                    nc.sync.dma_start(
                        q_sb[:], q_d.rearrange("(k p) d -> p k d", p=128))
                q_bf = bigp.tile([128, KC, D], bf16, tag="q_bf")
                nc.vector.tensor_copy(q_bf[:], q_sb[:])

                c_sb = []
                if o["memset_in"]:
                    for jj in range(4):
                        t_ = bigp.tile([128, 4, D], f32, tag=f"c_sb{jj}")
                        nc.gpsimd.memset(t_[:], 0.02)
                        c_sb.append(t_)
                elif o["contig_in"] and o["c_onebuf"]:
                    crs = c_d.rearrange("(p j) d -> p j d", j=TT)
                    c_dt = f32r if o["c_f32r"] else f32
                    c_big = bigp.tile([128, TT, D], c_dt, tag="c_big")
                    for _j in range(TT):
                        if o["c_f32r"]:
                            nc.sync.dma_start(c_big[:, _j, :],
                                              crs[:, _j, :].bitcast(f32r))
                        else:
                            nc.sync.dma_start(c_big[:, _j, :], crs[:, _j, :])
                elif o["contig_in"] and o["fine_c"]:
                    crs = c_d.rearrange("(p j) d -> p j d", j=TT)
                    c_fine = []
                    for _j in range(TT):
                        cf = bigp.tile([128, D], f32, tag=f"cin{_j}")
                        nc.sync.dma_start(cf[:], crs[:, _j, :])
                        c_fine.append(cf)
                elif o["contig_in"]:
                    crs = c_d.rearrange("(p j) d -> p j d", j=TT)
                    if o["split_in"]:
                        for jj in range(4):
                            t_ = bigp.tile([128, 4, D], f32, tag=f"c_sb{jj}")
                            for jr in range(4):
                                nc.sync.dma_start(
                                    t_[:, jr:jr + 1, :],
                                    crs[:, 4 * jj + jr:4 * jj + jr + 1, :])
                            c_sb.append(t_)
                    else:
                        for jj in range(4):
                            t_ = bigp.tile([128, 4, D], f32, tag=f"c_sb{jj}")
                            nc.sync.dma_start(t_[:],
                                              crs[:, 4 * jj:4 * jj + 4, :])
                            c_sb.append(t_)
                else:
                    for jj in range(4):
                        t_ = bigp.tile([128, 4, D], f32, tag=f"c_sb{jj}")
                        nc.sync.dma_start(
                            t_[:],
                            c_d[jj * 512:(jj + 1) * 512, :].rearrange(
                                "(j p) d -> p j d", p=128))
                        c_sb.append(t_)

                if o["contig_in"]:
                    ors = out_d.rearrange("(p j) w -> p j w", j=TT)

                    def out_ap(j, sl):
                        return ors[:, j, sl]
                else:
                    def out_ap(j, sl):
                        return out_d[j * 128:(j + 1) * 128, sl]

                if o["c_f32r"]:
                    assert o["c_onebuf"], "c_f32r requires c_onebuf"

                def c_tile(j):
                    if o["contig_in"] and o["c_onebuf"]:
                        return c_big[:, j]
                    if o["contig_in"] and o["fine_c"]:
                        return c_fine[j]
                    jj_, jr_ = divmod(j, 4)
                    return c_sb[jj_][:, jr_]

                def c_f32(ap):
                    # f32 view of c for DVE/Pool/DMA when stored as f32r
                    return ap.bitcast(f32) if o["c_f32r"] else ap

                _out_n = [0]

                def out_dma(j, sl, src):
                    if o["skip_out"]:
                        return
                    _out_n[0] += 1
                    if o["out_alt"]:
                        eng = nc.gpsimd if _out_n[0] % 2 else nc.sync
                    elif o["out_split"] and _out_n[0] % 2:
                        eng = nc.scalar
                    else:
                        eng = out_eng
                    eng.dma_start(out_ap(j, sl), src)

                if o["dma_pair"]:
                    for j in range(TT):
                        jj, jr = divmod(j, 4)
                        cj = c_sb[jj][:, jr]
                        out_dma(j, slice(0, 512), cj[:])
                        out_dma(j, slice(512, 2048),
                                c_sb[jj][:].rearrange("p a d -> p (a d)")
                                [:, 0:1536])
                    continue

                def copy_op(dst, src):
                    if o["act_copies"] == "dve":
                        nc.vector.tensor_copy(dst, src)
                    else:
                        nc.scalar.copy(dst, src)

                if o["o1_batch"]:
                    if not o["skip_out"]:
                        out_eng.dma_start(ors[:, :, 0:512], c_f32(c_big[:]))
                elif o["early_cout"] and not o["dma_pair"]:
                    for j in range(TT):
                        out_dma(j, slice(0, 512), c_tile(j))

                # Q^T, qa = Q^T * wcq + wc
                qt = bigp.tile([128, KC, I], f32, tag="qt")
                qa = bigp.tile([128, KC, I], f32r, tag="qa")
                for k in range(KC):
                    pt = ps_tr.tile([128, I], f32, tag="ps_tr")
                    for ik in range(KC):
                        nc.tensor.transpose(
                            pt[:, ik * 128:(ik + 1) * 128],
                            q_sb[:, ik, k * 128:(k + 1) * 128],
                            ident_f[:])
                    copy_op(qt[:, k], pt[:])
                    nc.vector.tensor_scalar(
                        qa[:, k], pt[:], wcq_col[:, k:k + 1],
                        wc_col[:, k:k + 1], op0=ALU.mult, op1=ALU.add)

                # sq_row[1, I] = w_q^T Q^T
                ps_sq = ps_s.tile([1, I], f32, tag="ps_s")
                for k in range(KC):
                    nc.tensor.matmul(ps_sq[:], wq_col[:, k:k + 1], qt[:, k],
                                     start=(k == 0), stop=(k == KC - 1))
                sq_row = constp.tile([1, I], f32r, tag="sq_row")
                copy_op(sq_row[:], ps_sq[:])

                scratch1 = constp.tile([1, 1], f32, tag="scratch1")
                g = constp.tile([128, TT], f32r if o["c_f32r"] else f32,
                                tag="g")
                if o["q2c_inline"]:
                    psq2c = ps_acc.tile([1, D], f32, tag="ps_q2c")
                    psZ = ps_acc.tile([1, 1], f32, tag="ps_Z")
                mhat = constp.tile([128, TT], f32, tag="mhat")
                r_col = constp.tile([128, TT], f32, tag="r_col")
                rinv = constp.tile([128, TT], f32, tag="rinv")
                if o["fine_tiles"]:
                    et_tiles = []
                    for _j in range(TT):
                        et_j = bigp.tile([128, KC, 128], bf16,
                                         tag=f"et{_j}")
                        et_tiles.append(et_j)
                    r_tiles = []
                    ri_tiles = []
                    for _j in range(TT):
                        r_j = bigp.tile([128, 1], f32, tag=f"r{_j}")
                        r_tiles.append(r_j)
                        ri_j = bigp.tile([128, 1], f32, tag=f"ri{_j}")
                        ri_tiles.append(ri_j)
                else:
                    et = bigp.tile([128, KC, T], bf16, tag="et")

                # ---------------- phase 1: per row-tile ----------------------
                def do_mm2_epilogue(j, q2c_bc):
                    cj = c_tile(j)
                    pc = ps_mm2.tile([128, D], f32, tag="ps_mm2")
                    for ik in range(KC):
                        lhs_mm2 = (et_tiles[j][:, ik, :] if o["fine_tiles"]
                                   else et[:, ik, j * 128:(j + 1) * 128])
                        nc.tensor.matmul(pc[:], lhs_mm2, q_bf[:, ik],
                                         start=(ik == 0), stop=(ik == KC - 1))
                    if q2c_bc is None:
                        o_t = outp.tile([128, 1024], f32, tag="o23")
                        if o["act_copies"] == "dve":
                            nc.vector.tensor_scalar_mul(o_t[:, 0:512], pc[:],
                                                        (ri_tiles[j][:] if o["fine_tiles"] else rinv[:, j:j + 1]))
                        else:
                            nc.scalar.mul(o_t[:, 0:512], pc[:],
                                          (ri_tiles[j][:] if o["fine_tiles"]
                                           else rinv[:, j:j + 1]))
                        mul_e = (nc.gpsimd if o["mul_eng"] == "gpsimd"
                                 else nc.vector)
                        mul_e.tensor_mul(o_t[:, 512:1024], c_f32(cj[:]),
                                         o_t[:, 0:512])
                        if o["dup_dve"]:
                            nc.vector.tensor_mul(o_t[:, 512:1024],
                                                 c_f32(cj[:]),
                                                 o_t[:, 0:512])
                        out_dma(j, slice(512, 1536), o_t[:])
                    else:
                        o_t = outp.tile([128, 1536], f32, tag="o234")
                        if o["act_copies"] == "dve":
                            nc.vector.tensor_scalar_mul(o_t[:, 0:512], pc[:],
                                                        (ri_tiles[j][:] if o["fine_tiles"] else rinv[:, j:j + 1]))
                        else:
                            nc.scalar.mul(o_t[:, 0:512], pc[:],
                                          (ri_tiles[j][:] if o["fine_tiles"]
                                           else rinv[:, j:j + 1]))
                        nc.vector.tensor_mul(o_t[:, 512:1024], c_f32(cj[:]),
                                             o_t[:, 0:512])
                        nc.vector.tensor_mul(o_t[:, 1024:1536], c_f32(cj[:]),
                                             q2c_bc[:])
                        out_dma(j, slice(512, 2048), o_t[:])

                for j in range(TT):
                    cj = c_tile(j)  # [128, 512] fp32

                    # C^T for this tile
                    pt = ps_tr.tile([128, 512], f32, tag="ps_tr")
                    if o["c_f32r"]:
                        for k in range(KC):
                            nc.tensor.transpose(
                                pt[:, k * 128:(k + 1) * 128].bitcast(f32r),
                                cj[:, k * 128:(k + 1) * 128], ident_r)
                    else:
                        for k in range(KC):
                            nc.tensor.transpose(
                                pt[:, k * 128:(k + 1) * 128],
                                cj[:, k * 128:(k + 1) * 128], ident_f[:])
                    ct = workp.tile([128, 512], f32r, tag="ct")
                    if o["ct_eng"] == "act" or (o["alt_copies"] and j % 2 == 0):
                        nc.scalar.copy(ct[:], pt[:])
                    else:
                        nc.vector.tensor_copy(ct[:], pt[:])
                    if o["dup_dve"]:
                        nc.vector.tensor_copy(ct[:], pt[:])

                    # mm1: s' = c @ qa + 1*sq
                    ps = ps_s.tile([128, I], f32, tag="ps_s")
                    if o["dup_pe"]:
                        for k in range(KC):
                            nc.tensor.matmul(
                                ps[:], ct[:, k * 128:(k + 1) * 128],
                                qa[:, k], start=(k == 0), stop=False,
                                skip_group_check=True)
                        for k in range(KC):
                            nc.tensor.matmul(
                                ps[:], ct[:, k * 128:(k + 1) * 128],
                                qa[:, k], start=(k == 0), stop=False,
                                skip_group_check=True)
                    else:
                        for k in range(KC):
                            nc.tensor.matmul(
                                ps[:], ct[:, k * 128:(k + 1) * 128],
                                qa[:, k], start=(k == 0), stop=False)
                    nc.tensor.matmul(ps[:], ones_row[:], sq_row[:],
                                     start=False, stop=True)

                    nc.vector.reduce_max(mhat[:, j:j + 1], ps[:], axis=AX.X)

                    e_tile = workp.tile([128, I], bf16, tag="e")
                    r_dst = (r_tiles[j][:] if o["fine_tiles"]
                             else r_col[:, j:j + 1])
                    if o["exp_accum"]:
                        nc.scalar.activation(e_tile[:], ps[:], AF.Exp,
                                             accum_out=r_dst)
                    else:
                        nc.scalar.activation(e_tile[:], ps[:], AF.Exp)
                        nc.vector.reduce_sum(r_dst, e_tile[:], axis=AX.X)
                    if o["dup_act"]:
                        nc.scalar.activation(e_tile[:], ps[:], AF.Exp,
                                             accum_out=r_col[:, j:j + 1])
                    if o["fine_tiles"]:
                        nc.vector.reciprocal(ri_tiles[j][:], r_tiles[j][:])
                    elif o["batch_recip"]:
                        if j % 4 == 3:
                            nc.vector.reciprocal(rinv[:, j - 3:j + 1],
                                                 r_col[:, j - 3:j + 1])
                    else:
                        nc.vector.reciprocal(rinv[:, j:j + 1],
                                             r_col[:, j:j + 1])

                    # E^T into et[:, ik, j*128:...]
                    pe = ps_tr.tile([128, 512], bf16, tag="ps_tr")
                    for ik in range(KC):
                        nc.tensor.transpose(
                            pe[:, ik * 128:(ik + 1) * 128],
                            e_tile[:, ik * 128:(ik + 1) * 128], ident_b[:])
                    et_dst = (et_tiles[j][:] if o["fine_tiles"]
                              else et[:, :, j * 128:(j + 1) * 128])
                    if o["alt_copies"] and j % 2 == 1:
                        nc.vector.tensor_copy(
                            et_dst, pe[:].rearrange("p (a b) -> p a b", a=KC))
                    else:
                        copy_op(et_dst,
                                pe[:].rearrange("p (a b) -> p a b", a=KC))

                    for _b in range(o["bloat"]):
                        nc.vector.memset(scratch1[0:1, 0:1], 0.0)

                    if o["q2c_inline"]:
                        nc.scalar.activation(g[:, j:j + 1], mhat[:, j:j + 1],
                                             AF.Exp)
                        nc.tensor.matmul(psq2c[:], g[:, j:j + 1], cj[:],
                                         start=(j == 0), stop=(j == TT - 1),
                                         skip_group_check=True)
                        nc.tensor.matmul(psZ[:], g[:, j:j + 1], ones_col[:],
                                         start=(j == 0), stop=(j == TT - 1),
                                         skip_group_check=True)

                    # c block can go out as soon as loaded
                    if not o["early_cout"] and not o["o1_batch"]:
                        out_dma(j, slice(0, 512), c_f32(cj[:]))

                    if not o["two_pass"]:
                        do_mm2_epilogue(j, None)

                # ---------------- phase 2: q2c -------------------------------
                if not o["q2c_inline"]:
                    gsum = constp.tile([128, 1], f32, tag="gsum")
                    if o["g_accum"]:
                        nc.scalar.activation(g[:], mhat[:], AF.Exp,
                                             accum_out=gsum[:])
                    else:
                        nc.scalar.activation(g[:], mhat[:], AF.Exp)
                        nc.vector.reduce_sum(gsum[:], g[:], axis=AX.X)
                    psZ = ps_s.tile([1, 1], f32, tag="ps_s")
                    nc.tensor.matmul(psZ[:], ones_col[:], gsum[:],
                                     start=True, stop=True)
                    psq2c = ps_s.tile([1, D], f32, tag="ps_s")
                    for j in range(TT):
                        nc.tensor.matmul(psq2c[:], g[:, j:j + 1], c_tile(j),
                                         start=(j == 0), stop=(j == TT - 1))
                Zinv = constp.tile([1, 1], f32, tag="Zinv")
                nc.vector.reciprocal(Zinv[:], psZ[:])
                q2c_row = constp.tile([1, D], f32r if o["c_f32r"] else f32,
                                      tag="q2c_row")
                nc.vector.tensor_scalar_mul(q2c_row[:], psq2c[:], Zinv[:])

                psbc = ps_s.tile([128, D], f32, tag="ps_s")
                if o["c_f32r"]:
                    nc.tensor.matmul(psbc[:], ones_row[:], q2c_row[:],
                                     start=True, stop=True)
                else:
                    nc.tensor.matmul(psbc[:], ones_row_f[:], q2c_row[:],
                                     start=True, stop=True)
                q2c_bc = constp.tile([128, D], f32, tag="q2c_bc")
                copy_op(q2c_bc[:], psbc[:])

                # ---------------- phase 3 ------------------------------------
                if o["two_pass"]:
                    for j in range(TT):
                        do_mm2_epilogue(j, q2c_bc)
                else:
                    for j in range(TT):
                        jj, jr = divmod(j, 4)
                        if o["o4_split"]:
                            mul_e4 = nc.gpsimd if j % 2 else nc.vector
                        else:
                            mul_e4 = (nc.gpsimd if o["mul_eng"] == "gpsimd"
                                      else nc.vector)
                        o4 = outp.tile([128, D], f32, tag="o4")
                        mul_e4.tensor_mul(o4[:], c_f32(c_tile(j)[:]),
                                          q2c_bc[:])
                        out_dma(j, slice(1536, 2048), o4[:])

        if timing_mode:
            with tc.tile_pool(name="tickp", bufs=1) as tickp:
                tk = tickp.tile([1, 1], f32, tag="tick")
                nc.vector.memset(tk[:], 1.0)
                nc.sync.dma_start(tick_d[:], tk[:])

    nc.compile()
    return nc


# Default kernel: the v3 s-transposed formulation (sq via the matmul path;
# tensor_tensor_reduce and Act-engine f32r writes fault on HW), with input
# DMA dispatch split across SP/Act rings and depth-4 work/out pools.
KERNEL_OPTS = {"v3": True, "sq_mm": True, "in_alt": True,
               "bufs_work": 4, "bufs_out": 4}


def _get_built():
    global _BUILT
    if _BUILT is None:
        _BUILT = _build(opts=KERNEL_OPTS)
    return _BUILT


def kernel(c, q, w_c, b_c, w_q, b_q, w_cq, b_cq):
    """Full inputs in, full output out. Data-parallel over batch on 8 cores.

    Biases cancel mathematically (softmax shift invariance), so b_* are
    accepted but unused.
    """
    from concourse import bass_utils

    nc = _get_built()
    c = np.ascontiguousarray(np.asarray(c, dtype=np.float32))
    q = np.ascontiguousarray(np.asarray(q, dtype=np.float32))
    wc = np.ascontiguousarray(np.asarray(w_c, dtype=np.float32))
    wq = np.ascontiguousarray(np.asarray(w_q, dtype=np.float32))
    wcq = np.ascontiguousarray(np.asarray(w_cq, dtype=np.float32))

    in_maps = [
        {"c": c[b], "q": q[b], "wc": wc, "wq": wq, "wcq": wcq}
        for b in range(N_CORES)
    ]
    res = bass_utils.run_bass_kernel_spmd(
        nc, in_maps, core_ids=list(range(N_CORES)))
    return np.stack([res.results[b]["out"] for b in range(N_CORES)])



# revision 67
# speedup vs baseline: 1.4134x; 1.4134x over previous
"""Trainium2 Bass kernel for nn_AttentionFlow (BiDAF-style attention flow).

Math (per batch b, all biases cancel):
  s[t,i]   = <c_t,w_c> + <q_i,w_q> + <c_t*q_i, w_cq>  (+ biases)
  a        = softmax_i(s)          -> c2q = a @ q
  beta     = softmax_t(max_i s)    -> q2c = beta^T c
  out      = [c | c2q | c*c2q | c*q2c]

Key identities:
  * softmax_i(s[t,:]) is invariant to the per-row term sc[t] and all biases.
  * sc is folded into the matmul weights: qa[d,i] = q^T[d,i]*w_cq[d]+w_c[d].
  * exp(max_i s) = max_i exp(s), so beta's numerator comes from a max over
    the already-exponentiated E with no extra exp.
  * t and i orderings are arbitrary (softmax/sums are order-invariant and
    outputs are re-addressed by AP); i is stored as i = 4p + k.

Shipped kernel (_build_v3, KERNEL_OPTS): computes s TRANSPOSED per pair of
128-row tiles (256 t columns, which keeps f32r matmuls at full rate):
  s^T[i,t] = sum_d qa[d,i] C^T[d,t];  E^T = exp(s^T + sq[i]) via the Act
engine with sq as a per-partition bias (free); c2q and the row sums r come
from mm2 with E^T chunks stationary against q rows in natural layout plus a
ones column — no E transpose and no sq matmul. beta's g = max_i E^T via a
bf16 max tree + one 128-wide PE transpose. c/q live in SBUF as f32r
(DMA-bitcast) so every matmul and PE transpose runs at f32r rate; the
f32->f32r "rounded producer" rule is satisfied because DMA and DVE outputs
count as rounded (Act does NOT - it faults on HW - and tensor_tensor_reduce
faults outright).

Perf model (per core): 21MB HBM traffic (c 4MB in, q 1MB in, out 16MB) at
~360GB/s/core means a ~58us DMA roofline; engine busy (cost model) is
PE 40us, DVE 40us, SP 33us, Act 31us, Pool 26us - all below the roofline,
so the kernel is DMA-bound when the shared terminal is quiet and degrades
proportionally to HBM contention. Output descriptors are 2KB+/partition,
o1 (=c) goes out as one 4MB DMA, out-DMA dispatch alternates SP/Pool rings
and input dispatch alternates SP/Act rings so no single sequencer
serializes the stream.

Sharding: data-parallel over batch, one batch element per NeuronCore (8).
"""

import numpy as np

N_CORES = 8
T, I, D = 2048, 512, 512
TT = T // 128  # 16 row tiles
KC = 4         # 128-chunks of D (and of I)

DEFAULT_OPTS = dict(contig_in=True, out_ring="sync", two_pass=False,
                    skip_out=False, memset_in=False, dma_pair=False,
                    bufs_work=3, bufs_out=3, out_split=False,
                    dup_pe=False, dup_dve=False, dup_act=False, bloat=0,
                    act_copies="act", exp_accum=True, g_accum=True,
                    mul_eng="gpsimd", batch_recip=False, alt_copies=False,
                    split_in=True, q2c_inline=False, o4_split=True,
                    ps_tr_bufs=2, ct_eng="dve", early_cout=False,
                    ps_s_bufs=2, ps_mm2_bufs=2, fine_tiles=True, fine_c=False,
                    c_onebuf=True, o1_batch=True, tr_f32r=False,
                    q2c_f32r=False, out_alt=True, c_f32r=True)

_BUILT = None


def _build_v3(reps=1, timing_mode=False, opts=None):
    """s-transposed formulation.

    Per pair of 128-row tiles (256 t-columns, satisfying the f32r
    ap>=256 full-rate rule):
      s^T[i,t] = sum_d qa[d,i] * C^T[d,t]   (qa = Q^T*wcq + wc, folds sc)
      E^T = exp(s^T + sq[i])                (sq per-partition Act bias)
      g[t] = max_i E^T[i,t]  (= exp(max_i s) by monotonicity; bf16 max
             tree + one 128-wide PE transpose + free-axis reduce_max)
      c2q-row r[t] and c2q via mm2 with E^T chunks as stationary weights
             (rhs = q rows natural layout; ones column gives r).
    No E transpose, no sq matmul, q2c contraction in f32r.
    """
    import concourse.tile as tile
    from concourse import bacc, mybir
    from concourse.masks import make_identity

    o = dict(v3_mul_split=True, out_alt=True, o1_batch=True,
             seq_pr=False, sq_mm=False, in_alt=False, bufs_work=3,
             bufs_out=3, ct_alt=False, setup_pool=False, tail_dve=False,
             q2c_il=False, o1_split=1)
    if opts:
        o.update(opts)

    f32 = mybir.dt.float32
    f32r = mybir.dt.float32r
    bf16 = mybir.dt.bfloat16
    AF = mybir.ActivationFunctionType
    AX = mybir.AxisListType
    ALU = mybir.AluOpType
    IC = 4

    nc = bacc.Bacc("TRN2", target_bir_lowering=False, debug=False,
                   num_devices=N_CORES)
    c_d = nc.dram_tensor("c", [T, D], f32, kind="ExternalInput").ap()
    q_d = nc.dram_tensor("q", [I, D], f32, kind="ExternalInput").ap()
    wc_d = nc.dram_tensor("wc", [D], f32, kind="ExternalInput").ap()
    wq_d = nc.dram_tensor("wq", [D], f32, kind="ExternalInput").ap()
    wcq_d = nc.dram_tensor("wcq", [D], f32, kind="ExternalInput").ap()
    out_kind = "Internal" if timing_mode else "ExternalOutput"
    out_d = nc.dram_tensor("out", [T, 4 * D], f32, kind=out_kind).ap()
    tick_d = (nc.dram_tensor("tick", [1, 1], f32, kind="ExternalOutput").ap()
              if timing_mode else None)

    with tile.TileContext(nc) as tc:
        with (
            tc.tile_pool(name="const", bufs=1) as constp,
            tc.tile_pool(name="big", bufs=1) as bigp,
            tc.tile_pool(name="work", bufs=o["bufs_work"]) as workp,
            tc.tile_pool(name="outp", bufs=o["bufs_out"]) as outp,
            tc.tile_pool(name="ps_tr", bufs=2, space="PSUM") as ps_tr,
            tc.tile_pool(name="ps_s", bufs=2 if o["q2c_il"] else 3,
                         space="PSUM") as ps_s,
            tc.tile_pool(name="ps_mm2", bufs=2, space="PSUM") as ps_mm2,
            tc.tile_pool(name="ps_sm", bufs=1, space="PSUM") as ps_sm,
            tc.tile_pool(name="ps_q2i", bufs=1, space="PSUM") as ps_q2i,
        ):
            for _rep in range(reps):
                # ---------------- setup --------------------------------------
                ident_f = constp.tile([128, 128], f32, tag="idf")
                make_identity(nc, ident_f[:])
                ident_b = constp.tile([128, 128], bf16, tag="idb")
                make_identity(nc, ident_b[:])
                ident_r = constp.tile([128, 128], f32r, tag="idr")
                nc.vector.tensor_copy(ident_r[:], ident_f[:])
                ones_row_f = constp.tile([1, 128], f32, tag="ones_row_f")
                nc.vector.memset(ones_row_f[:], 1.0)
                ones_row = constp.tile([1, 128], f32r, tag="ones_row")
                nc.vector.tensor_copy(ones_row[:], ones_row_f[:])
                ones_col = constp.tile([128, 1], f32, tag="ones_col")
                nc.vector.memset(ones_col[:], 1.0)
                ones_col_b = constp.tile([128, 1], bf16, tag="ones_col_b")
                nc.vector.memset(ones_col_b[:], 1.0)

                setup_eng = nc.gpsimd if o["setup_pool"] else nc.sync
                wcq_col = constp.tile([128, KC], f32, tag="wcq_col")
                setup_eng.dma_start(wcq_col[:],
                                    wcq_d.rearrange("(a b) -> b a", b=128))
                wc_col = constp.tile([128, KC], f32, tag="wc_col")
                setup_eng.dma_start(wc_col[:],
                                    wc_d.rearrange("(a b) -> b a", b=128))
                wq_row = constp.tile([1, D], f32, tag="wq_row")
                setup_eng.dma_start(wq_row[:],
                                    wq_d.rearrange("(a d) -> a d", a=1))

                # q in [i_part, d] with i = 4p+k; f32r so PE transposes and
                # setup matmuls run at full f32r rate
                q_sb = bigp.tile([128, KC, D], f32r, tag="q_sb")
                setup_eng.dma_start(
                    q_sb[:],
                    q_d.rearrange("(p k) d -> p k d", k=KC).bitcast(f32r))
                q_bf = bigp.tile([128, KC, D], bf16, tag="q_bf")
                if o["setup_pool"]:
                    nc.gpsimd.tensor_copy(q_bf[:], q_sb[:].bitcast(f32))
                else:
                    nc.vector.tensor_copy(q_bf[:], q_sb[:].bitcast(f32))

                # c in f32r, one contiguous buffer, fine-grained loads
                crs = c_d.rearrange("(p j) d -> p j d", j=TT)
                c_big = bigp.tile([128, TT, D], f32r, tag="c_big")
                for _j in range(TT):
                    in_eng = (nc.scalar if (o["in_alt"] and _j % 2)
                              else nc.sync)
                    in_eng.dma_start(c_big[:, _j, :],
                                     crs[:, _j, :].bitcast(f32r))

                ors = out_d.rearrange("(p j) w -> p j w", j=TT)

                _out_n = [0]

                def out_dma(j, sl, src):
                    _out_n[0] += 1
                    eng = (nc.gpsimd if (o["out_alt"] and _out_n[0] % 2)
                           else nc.sync)
                    eng.dma_start(ors[:, j, sl], src)

                # o1 = c passthrough, one (or a few) big DMAs
                if o["o1_batch"]:
                    nsp = o["o1_split"]
                    w = TT // nsp
                    for sp in range(nsp):
                        nc.sync.dma_start(
                            ors[:, sp * w:(sp + 1) * w, 0:512],
                            c_big[:, sp * w:(sp + 1) * w, :].bitcast(f32))

                sq_col = constp.tile([128, KC], f32, tag="sq_col")
                if not o["sq_mm"]:
                    # wq broadcast to all partitions (via PE), then per-row
                    # dots: sq_col[p, k] = <q[4p+k, :], wq>
                    ps_bc = ps_mm2.tile([128, D], f32, tag="pc")
                    nc.tensor.matmul(ps_bc[:], ones_row_f[:], wq_row[:],
                                     start=True, stop=True)
                    wq_bc = constp.tile([128, D], f32, tag="wq_bc")
                    nc.scalar.copy(wq_bc[:], ps_bc[:])
                    sq_scr = constp.tile([128, D], f32, tag="sq_scr")
                    for k in range(KC):
                        nc.vector.tensor_tensor_reduce(
                            sq_scr[:], q_sb[:, k], wq_bc[:], 1.0, 0.0,
                            ALU.mult, ALU.add, accum_out=sq_col[:, k:k + 1])

                # qa[d, i] = Q^T * wcq + wc  (via PE transposes of q)
                qa = bigp.tile([128, KC, I], f32r, tag="qa")
                if o["sq_mm"]:
                    qt = bigp.tile([128, KC, I], f32r, tag="qt")
                else:
                    qt = None
                for k in range(KC):
                    pt = ps_mm2.tile([128, I], f32, tag="pc")
                    for ik in range(KC):
                        nc.tensor.transpose(
                            pt[:, ik * 128:(ik + 1) * 128].bitcast(f32r),
                            q_sb[:, ik, k * 128:(k + 1) * 128],
                            ident_r[:])
                    if o["sq_mm"]:
                        nc.vector.tensor_copy(qt[:, k], pt[:])
                    nc.vector.tensor_scalar(
                        qa[:, k], pt[:], wcq_col[:, k:k + 1],
                        wc_col[:, k:k + 1], op0=ALU.mult, op1=ALU.add)

                if o["sq_mm"]:
                    # sq_row = wq^T Q^T, then 4 thin transposes into sq_col
                    wq_col = constp.tile([128, KC], f32r, tag="wq_col")
                    nc.sync.dma_start(
                        wq_col[:],
                        wq_d.rearrange("(a b) -> b a", b=128).bitcast(f32r))
                    ps_sq = ps_mm2.tile([1, I], f32, tag="pc")
                    for k in range(KC):
                        nc.tensor.matmul(ps_sq[:], wq_col[:, k:k + 1],
                                         qt[:, k], start=(k == 0),
                                         stop=(k == KC - 1))
                    sq_row = constp.tile([1, I], f32, tag="sq_row")
                    nc.scalar.copy(sq_row[:], ps_sq[:])
                    sq_ps = ps_sm.tile([128, KC], f32, tag="pr")
                    for ic in range(IC):
                        nc.tensor.transpose(
                            sq_ps[:, ic:ic + 1],
                            sq_row[0:1, ic * 128:(ic + 1) * 128],
                            ones_row_f[0:1, 0:1])
                    nc.vector.tensor_copy(sq_col[:], sq_ps[:])

                g = constp.tile([128, TT], f32r, tag="g")
                ri_tiles = []
                for _j in range(TT):
                    ri_j = bigp.tile([128, 1], f32, tag=f"ri{_j}")
                    ri_tiles.append(ri_j)

                if o["q2c_il"]:
                    psq2c_il = ps_q2i.tile([1, D], f32, tag="q2i")

                # ---------------- phase 1: per pair of tiles -----------------
                for m in range(TT // 2):
                    j0 = 2 * m

                    # C^T for the pair: [d_part, 256 t]
                    ct = workp.tile([128, KC, 256], f32r, tag="ct")
                    for k in range(KC):
                        ptk = ps_tr.tile([128, 256], f32, tag="ptk")
                        for jj in range(2):
                            nc.tensor.transpose(
                                ptk[:, jj * 128:(jj + 1) * 128]
                                .bitcast(f32r),
                                c_big[:, j0 + jj,
                                      k * 128:(k + 1) * 128],
                                ident_r[:])
                        if o["ct_alt"] and k % 2:
                            nc.gpsimd.tensor_copy(ct[:, k], ptk[:])
                        else:
                            nc.vector.tensor_copy(ct[:, k], ptk[:])

                    # mm1 per i-chunk + exp + bf16 max tree
                    et2 = workp.tile([128, IC, 256], bf16, tag="et2")
                    m4e = workp.tile([128, 256], bf16, tag="m4e")
                    for ic in range(IC):
                        psT = ps_s.tile([128, 256], f32, tag="psT")
                        for k in range(KC):
                            nc.tensor.matmul(
                                psT[:], qa[:, k, ic * 128:(ic + 1) * 128],
                                ct[:, k], start=(k == 0), stop=(k == KC - 1))
                        nc.scalar.activation(et2[:, ic, :], psT[:], AF.Exp,
                                             bias=sq_col[:, ic:ic + 1])
                        if ic == 0:
                            nc.vector.tensor_copy(m4e[:], et2[:, 0, :])
                        else:
                            nc.vector.tensor_tensor(
                                m4e[:], m4e[:], et2[:, ic, :], ALU.max)

                    # per tile: g column, mm2, epilogue
                    for jj in range(2):
                        j = j0 + jj
                        mt = ps_tr.tile([128, 128], bf16, tag="ptk")
                        nc.tensor.transpose(
                            mt[:], m4e[:, jj * 128:(jj + 1) * 128],
                            ident_b[:])
                        nc.vector.reduce_max(g[:, j:j + 1], mt[:], axis=AX.X)
                        if o["q2c_il"]:
                            nc.tensor.matmul(psq2c_il[:], g[:, j:j + 1],
                                             c_big[:, j], start=(j == 0),
                                             stop=(j == TT - 1),
                                             skip_group_check=True)

                        pc = ps_mm2.tile([128, 512], f32, tag="pc")
                        pr = ps_sm.tile([128, 1], f32, tag="pr")
                        if o["seq_pr"]:
                            for ic in range(IC):
                                nc.tensor.matmul(
                                    pc[:], et2[:, ic, jj * 128:(jj + 1) * 128],
                                    q_bf[:, ic], start=(ic == 0),
                                    stop=(ic == IC - 1))
                            for ic in range(IC):
                                nc.tensor.matmul(
                                    pr[:], et2[:, ic, jj * 128:(jj + 1) * 128],
                                    ones_col_b[:], start=(ic == 0),
                                    stop=(ic == IC - 1))
                        else:
                            for ic in range(IC):
                                lhs = et2[:, ic, jj * 128:(jj + 1) * 128]
                                nc.tensor.matmul(pc[:], lhs, q_bf[:, ic],
                                                 start=(ic == 0),
                                                 stop=(ic == IC - 1))
                                nc.tensor.matmul(pr[:], lhs, ones_col_b[:],
                                                 start=(ic == 0),
                                                 stop=(ic == IC - 1),
                                                 skip_group_check=True)
                        nc.vector.reciprocal(ri_tiles[j][:], pr[:])
                        o_t = outp.tile([128, 1024], f32, tag="o23")
                        nc.scalar.mul(o_t[:, 0:512], pc[:], ri_tiles[j][:])
                        mul_e = (nc.gpsimd if (o["v3_mul_split"] and j % 2)
                                 else nc.vector)
                        mul_e.tensor_mul(o_t[:, 512:1024],
                                         c_big[:, j].bitcast(f32),
                                         o_t[:, 0:512])
                        out_dma(j, slice(512, 1536), o_t[:])

                # ---------------- phase 2: q2c -------------------------------
                gsum = constp.tile([128, 1], f32, tag="gsum")
                nc.vector.reduce_sum(gsum[:], g[:], axis=AX.X)
                psZ = ps_sm.tile([1, 1], f32, tag="pr")
                nc.tensor.matmul(psZ[:], ones_col[:], gsum[:],
                                 start=True, stop=True)
                if o["q2c_il"]:
                    psq2c = psq2c_il
                else:
                    psq2c = ps_mm2.tile([1, D], f32, tag="pc")
                    for j in range(TT):
                        nc.tensor.matmul(psq2c[:], g[:, j:j + 1],
                                         c_big[:, j], start=(j == 0),
                                         stop=(j == TT - 1))
                Zinv = constp.tile([1, 1], f32, tag="Zinv")
                nc.vector.reciprocal(Zinv[:], psZ[:])
                q2c_row = constp.tile([1, D], f32r, tag="q2c_row")
                nc.vector.tensor_scalar_mul(q2c_row[:], psq2c[:], Zinv[:])
                psbc = ps_mm2.tile([128, D], f32, tag="pc")
                nc.tensor.matmul(psbc[:], ones_row[:], q2c_row[:],
                                 start=True, stop=True)
                q2c_bc = constp.tile([128, D], f32, tag="q2c_bc")
                nc.scalar.copy(q2c_bc[:], psbc[:])

                # ---------------- phase 3: o4 --------------------------------
                for j in range(TT):
                    if o["tail_dve"]:
                        # Pool muls are ~2.3x slower; weight toward DVE
                        mul_e4 = nc.gpsimd if j % 3 == 2 else nc.vector
                    else:
                        mul_e4 = nc.gpsimd if j % 2 else nc.vector
                    o4 = outp.tile([128, D], f32, tag="o4")
                    mul_e4.tensor_mul(o4[:], c_big[:, j].bitcast(f32),
                                      q2c_bc[:])
                    out_dma(j, slice(1536, 2048), o4[:])

        if timing_mode:
            with tc.tile_pool(name="tickp", bufs=1) as tickp:
                tk = tickp.tile([1, 1], f32, tag="tick")
                nc.vector.memset(tk[:], 1.0)
                nc.sync.dma_start(tick_d[:], tk[:])

    nc.compile()
    return nc


def _build(reps=1, timing_mode=False, opts=None):
    if opts and opts.get("v3"):
        o2 = {k: v for k, v in opts.items() if k != "v3"}
        return _build_v3(reps, timing_mode, o2)
    import concourse.tile as tile
    from concourse import bacc, mybir
    from concourse.masks import make_identity

    o = dict(DEFAULT_OPTS)
    if opts:
        o.update(opts)

    f32 = mybir.dt.float32
    f32r = mybir.dt.float32r
    bf16 = mybir.dt.bfloat16
    AF = mybir.ActivationFunctionType
    AX = mybir.AxisListType
    ALU = mybir.AluOpType

    nc = bacc.Bacc("TRN2", target_bir_lowering=False, debug=False,
                   num_devices=N_CORES)
    c_d = nc.dram_tensor("c", [T, D], f32, kind="ExternalInput").ap()
    q_d = nc.dram_tensor("q", [I, D], f32, kind="ExternalInput").ap()
    wc_d = nc.dram_tensor("wc", [D], f32, kind="ExternalInput").ap()
    wq_d = nc.dram_tensor("wq", [D], f32, kind="ExternalInput").ap()
    wcq_d = nc.dram_tensor("wcq", [D], f32, kind="ExternalInput").ap()
    out_kind = "Internal" if timing_mode else "ExternalOutput"
    out_d = nc.dram_tensor("out", [T, 4 * D], f32, kind=out_kind).ap()
    tick_d = (nc.dram_tensor("tick", [1, 1], f32, kind="ExternalOutput").ap()
              if timing_mode else None)

    out_eng = {"sync": nc.sync, "scalar": nc.scalar, "gpsimd": nc.gpsimd,
               "vector": nc.vector}[o["out_ring"]]

    with tile.TileContext(nc) as tc:
        with (
            tc.tile_pool(name="const", bufs=1) as constp,
            tc.tile_pool(name="big", bufs=1) as bigp,
            tc.tile_pool(name="work", bufs=o["bufs_work"]) as workp,
            tc.tile_pool(name="outp", bufs=o["bufs_out"]) as outp,
            tc.tile_pool(name="ps_tr", bufs=o["ps_tr_bufs"],
                         space="PSUM") as ps_tr,
            tc.tile_pool(name="ps_acc", bufs=1, space="PSUM") as ps_acc,
            tc.tile_pool(name="ps_s", bufs=o["ps_s_bufs"],
                         space="PSUM") as ps_s,
            tc.tile_pool(name="ps_mm2", bufs=o["ps_mm2_bufs"],
                         space="PSUM") as ps_mm2,
        ):
            for _rep in range(reps):
                # ---------------- phase 0 -----------------------------------
                ident_f = constp.tile([128, 128], f32, tag="idf")
                make_identity(nc, ident_f[:])
                ident_b = constp.tile([128, 128], bf16, tag="idb")
                make_identity(nc, ident_b[:])

                if o["c_f32r"]:
                    ident_rt = constp.tile([128, 128], f32r, tag="idr")
                    nc.vector.tensor_copy(ident_rt[:], ident_f[:])
                    ident_r = ident_rt[:]
                ones_row_f = constp.tile([1, 128], f32, tag="ones_row_f")
                nc.vector.memset(ones_row_f[:], 1.0)
                ones_row = constp.tile([1, 128], f32r, tag="ones_row")
                nc.vector.tensor_copy(ones_row[:], ones_row_f[:])
                ones_col = constp.tile([128, 1], f32, tag="ones_col")
                nc.vector.memset(ones_col[:], 1.0)

                wcq_col = constp.tile([128, KC], f32, tag="wcq_col")
                nc.sync.dma_start(wcq_col[:],
                                  wcq_d.rearrange("(a b) -> b a", b=128))
                wc_col = constp.tile([128, KC], f32, tag="wc_col")
                nc.sync.dma_start(wc_col[:],
                                  wc_d.rearrange("(a b) -> b a", b=128))
                wq_col = constp.tile([128, KC], f32, tag="wq_col")
                nc.sync.dma_start(wq_col[:],
                                  wq_d.rearrange("(a b) -> b a", b=128))

                q_sb = bigp.tile([128, KC, D], f32, tag="q_sb")
                if o["memset_in"]:
                    nc.gpsimd.memset(q_sb[:], 0.01)
                elif o["contig_in"]:
                    nc.sync.dma_start(
                        q_sb[:], q_d.rearrange("(p k) d -> p k d", k=KC))
                else:
                    nc.sync.dma_start(
                        q_sb[:], q_d.rearrange("(k p) d -> p k d", p=128))
                q_bf = bigp.tile([128, KC, D], bf16, tag="q_bf")
                nc.vector.tensor_copy(q_bf[:], q_sb[:])

                c_sb = []
                if o["memset_in"]:
                    for jj in range(4):
                        t_ = bigp.tile([128, 4, D], f32, tag=f"c_sb{jj}")
                        nc.gpsimd.memset(t_[:], 0.02)
                        c_sb.append(t_)
                elif o["contig_in"] and o["c_onebuf"]:
                    crs = c_d.rearrange("(p j) d -> p j d", j=TT)
                    c_dt = f32r if o["c_f32r"] else f32
                    c_big = bigp.tile([128, TT, D], c_dt, tag="c_big")
                    for _j in range(TT):
                        if o["c_f32r"]:
                            nc.sync.dma_start(c_big[:, _j, :],
                                              crs[:, _j, :].bitcast(f32r))
                        else:
                            nc.sync.dma_start(c_big[:, _j, :], crs[:, _j, :])
                elif o["contig_in"] and o["fine_c"]:
                    crs = c_d.rearrange("(p j) d -> p j d", j=TT)
                    c_fine = []
                    for _j in range(TT):
                        cf = bigp.tile([128, D], f32, tag=f"cin{_j}")
                        nc.sync.dma_start(cf[:], crs[:, _j, :])
                        c_fine.append(cf)
                elif o["contig_in"]:
                    crs = c_d.rearrange("(p j) d -> p j d", j=TT)
                    if o["split_in"]:
                        for jj in range(4):
                            t_ = bigp.tile([128, 4, D], f32, tag=f"c_sb{jj}")
                            for jr in range(4):
                                nc.sync.dma_start(
                                    t_[:, jr:jr + 1, :],
                                    crs[:, 4 * jj + jr:4 * jj + jr + 1, :])
                            c_sb.append(t_)
                    else:
                        for jj in range(4):
                            t_ = bigp.tile([128, 4, D], f32, tag=f"c_sb{jj}")
                            nc.sync.dma_start(t_[:],
                                              crs[:, 4 * jj:4 * jj + 4, :])
                            c_sb.append(t_)
                else:
                    for jj in range(4):
                        t_ = bigp.tile([128, 4, D], f32, tag=f"c_sb{jj}")
                        nc.sync.dma_start(
                            t_[:],
                            c_d[jj * 512:(jj + 1) * 512, :].rearrange(
                                "(j p) d -> p j d", p=128))
                        c_sb.append(t_)

                if o["contig_in"]:
                    ors = out_d.rearrange("(p j) w -> p j w", j=TT)

                    def out_ap(j, sl):
                        return ors[:, j, sl]
                else:
                    def out_ap(j, sl):
                        return out_d[j * 128:(j + 1) * 128, sl]

                if o["c_f32r"]:
                    assert o["c_onebuf"], "c_f32r requires c_onebuf"

                def c_tile(j):
                    if o["contig_in"] and o["c_onebuf"]:
                        return c_big[:, j]
                    if o["contig_in"] and o["fine_c"]:
                        return c_fine[j]
                    jj_, jr_ = divmod(j, 4)
                    return c_sb[jj_][:, jr_]

                def c_f32(ap):
                    # f32 view of c for DVE/Pool/DMA when stored as f32r
                    return ap.bitcast(f32) if o["c_f32r"] else ap

                _out_n = [0]

                def out_dma(j, sl, src):
                    if o["skip_out"]:
                        return
                    _out_n[0] += 1
                    if o["out_alt"]:
                        eng = nc.gpsimd if _out_n[0] % 2 else nc.sync
                    elif o["out_split"] and _out_n[0] % 2:
                        eng = nc.scalar
                    else:
                        eng = out_eng
                    eng.dma_start(out_ap(j, sl), src)

                if o["dma_pair"]:
                    for j in range(TT):
                        jj, jr = divmod(j, 4)
                        cj = c_sb[jj][:, jr]
                        out_dma(j, slice(0, 512), cj[:])
                        out_dma(j, slice(512, 2048),
                                c_sb[jj][:].rearrange("p a d -> p (a d)")
                                [:, 0:1536])
                    continue

                def copy_op(dst, src):
                    if o["act_copies"] == "dve":
                        nc.vector.tensor_copy(dst, src)
                    else:
                        nc.scalar.copy(dst, src)

                if o["o1_batch"]:
                    if not o["skip_out"]:
                        out_eng.dma_start(ors[:, :, 0:512], c_f32(c_big[:]))
                elif o["early_cout"] and not o["dma_pair"]:
                    for j in range(TT):
                        out_dma(j, slice(0, 512), c_tile(j))

                # Q^T, qa = Q^T * wcq + wc
                qt = bigp.tile([128, KC, I], f32, tag="qt")
                qa = bigp.tile([128, KC, I], f32r, tag="qa")
                for k in range(KC):
                    pt = ps_tr.tile([128, I], f32, tag="ps_tr")
                    for ik in range(KC):
                        nc.tensor.transpose(
                            pt[:, ik * 128:(ik + 1) * 128],
                            q_sb[:, ik, k * 128:(k + 1) * 128],
                            ident_f[:])
                    copy_op(qt[:, k], pt[:])
                    nc.vector.tensor_scalar(
                        qa[:, k], pt[:], wcq_col[:, k:k + 1],
                        wc_col[:, k:k + 1], op0=ALU.mult, op1=ALU.add)

                # sq_row[1, I] = w_q^T Q^T
                ps_sq = ps_s.tile([1, I], f32, tag="ps_s")
                for k in range(KC):
                    nc.tensor.matmul(ps_sq[:], wq_col[:, k:k + 1], qt[:, k],
                                     start=(k == 0), stop=(k == KC - 1))
                sq_row = constp.tile([1, I], f32r, tag="sq_row")
                copy_op(sq_row[:], ps_sq[:])

                scratch1 = constp.tile([1, 1], f32, tag="scratch1")
                g = constp.tile([128, TT], f32r if o["c_f32r"] else f32,
                                tag="g")
                if o["q2c_inline"]:
                    psq2c = ps_acc.tile([1, D], f32, tag="ps_q2c")
                    psZ = ps_acc.tile([1, 1], f32, tag="ps_Z")
                mhat = constp.tile([128, TT], f32, tag="mhat")
                r_col = constp.tile([128, TT], f32, tag="r_col")
                rinv = constp.tile([128, TT], f32, tag="rinv")
                if o["fine_tiles"]:
                    et_tiles = []
                    for _j in range(TT):
                        et_j = bigp.tile([128, KC, 128], bf16,
                                         tag=f"et{_j}")
                        et_tiles.append(et_j)
                    r_tiles = []
                    ri_tiles = []
                    for _j in range(TT):
                        r_j = bigp.tile([128, 1], f32, tag=f"r{_j}")
                        r_tiles.append(r_j)
                        ri_j = bigp.tile([128, 1], f32, tag=f"ri{_j}")
                        ri_tiles.append(ri_j)
                else:
                    et = bigp.tile([128, KC, T], bf16, tag="et")

                # ---------------- phase 1: per row-tile ----------------------
                def do_mm2_epilogue(j, q2c_bc):
                    cj = c_tile(j)
                    pc = ps_mm2.tile([128, D], f32, tag="ps_mm2")
                    for ik in range(KC):
                        lhs_mm2 = (et_tiles[j][:, ik, :] if o["fine_tiles"]
                                   else et[:, ik, j * 128:(j + 1) * 128])
                        nc.tensor.matmul(pc[:], lhs_mm2, q_bf[:, ik],
                                         start=(ik == 0), stop=(ik == KC - 1))
                    if q2c_bc is None:
                        o_t = outp.tile([128, 1024], f32, tag="o23")
                        if o["act_copies"] == "dve":
                            nc.vector.tensor_scalar_mul(o_t[:, 0:512], pc[:],
                                                        (ri_tiles[j][:] if o["fine_tiles"] else rinv[:, j:j + 1]))
                        else:
                            nc.scalar.mul(o_t[:, 0:512], pc[:],
                                          (ri_tiles[j][:] if o["fine_tiles"]
                                           else rinv[:, j:j + 1]))
                        mul_e = (nc.gpsimd if o["mul_eng"] == "gpsimd"
                                 else nc.vector)
                        mul_e.tensor_mul(o_t[:, 512:1024], c_f32(cj[:]),
                                         o_t[:, 0:512])
                        if o["dup_dve"]:
                            nc.vector.tensor_mul(o_t[:, 512:1024],
                                                 c_f32(cj[:]),
                                                 o_t[:, 0:512])
                        out_dma(j, slice(512, 1536), o_t[:])
                    else:
                        o_t = outp.tile([128, 1536], f32, tag="o234")
                        if o["act_copies"] == "dve":
                            nc.vector.tensor_scalar_mul(o_t[:, 0:512], pc[:],
                                                        (ri_tiles[j][:] if o["fine_tiles"] else rinv[:, j:j + 1]))
                        else:
                            nc.scalar.mul(o_t[:, 0:512], pc[:],
                                          (ri_tiles[j][:] if o["fine_tiles"]
                                           else rinv[:, j:j + 1]))
                        nc.vector.tensor_mul(o_t[:, 512:1024], c_f32(cj[:]),
                                             o_t[:, 0:512])
                        nc.vector.tensor_mul(o_t[:, 1024:1536], c_f32(cj[:]),
                                             q2c_bc[:])
                        out_dma(j, slice(512, 2048), o_t[:])

                for j in range(TT):
                    cj = c_tile(j)  # [128, 512] fp32

                    # C^T for this tile
                    pt = ps_tr.tile([128, 512], f32, tag="ps_tr")
                    if o["c_f32r"]:
                        for k in range(KC):
                            nc.tensor.transpose(
                                pt[:, k * 128:(k + 1) * 128].bitcast(f32r),
                                cj[:, k * 128:(k + 1) * 128], ident_r)
                    else:
                        for k in range(KC):
                            nc.tensor.transpose(
                                pt[:, k * 128:(k + 1) * 128],
                                cj[:, k * 128:(k + 1) * 128], ident_f[:])
                    ct = workp.tile([128, 512], f32r, tag="ct")
                    if o["ct_eng"] == "act" or (o["alt_copies"] and j % 2 == 0):
                        nc.scalar.copy(ct[:], pt[:])
                    else:
                        nc.vector.tensor_copy(ct[:], pt[:])
                    if o["dup_dve"]:
                        nc.vector.tensor_copy(ct[:], pt[:])

                    # mm1: s' = c @ qa + 1*sq
                    ps = ps_s.tile([128, I], f32, tag="ps_s")
                    if o["dup_pe"]:
                        for k in range(KC):
                            nc.tensor.matmul(
                                ps[:], ct[:, k * 128:(k + 1) * 128],
                                qa[:, k], start=(k == 0), stop=False,
                                skip_group_check=True)
                        for k in range(KC):
                            nc.tensor.matmul(
                                ps[:], ct[:, k * 128:(k + 1) * 128],
                                qa[:, k], start=(k == 0), stop=False,
                                skip_group_check=True)
                    else:
                        for k in range(KC):
                            nc.tensor.matmul(
                                ps[:], ct[:, k * 128:(k + 1) * 128],
                                qa[:, k], start=(k == 0), stop=False)
                    nc.tensor.matmul(ps[:], ones_row[:], sq_row[:],
                                     start=False, stop=True)

                    nc.vector.reduce_max(mhat[:, j:j + 1], ps[:], axis=AX.X)

                    e_tile = workp.tile([128, I], bf16, tag="e")
                    r_dst = (r_tiles[j][:] if o["fine_tiles"]
                             else r_col[:, j:j + 1])
                    if o["exp_accum"]:
                        nc.scalar.activation(e_tile[:], ps[:], AF.Exp,
                                             accum_out=r_dst)
                    else:
                        nc.scalar.activation(e_tile[:], ps[:], AF.Exp)
                        nc.vector.reduce_sum(r_dst, e_tile[:], axis=AX.X)
                    if o["dup_act"]:
                        nc.scalar.activation(e_tile[:], ps[:], AF.Exp,
                                             accum_out=r_col[:, j:j + 1])
                    if o["fine_tiles"]:
                        nc.vector.reciprocal(ri_tiles[j][:], r_tiles[j][:])
                    elif o["batch_recip"]:
                        if j % 4 == 3:
                            nc.vector.reciprocal(rinv[:, j - 3:j + 1],
                                                 r_col[:, j - 3:j + 1])
                    else:
                        nc.vector.reciprocal(rinv[:, j:j + 1],
                                             r_col[:, j:j + 1])

                    # E^T into et[:, ik, j*128:...]
                    pe = ps_tr.tile([128, 512], bf16, tag="ps_tr")
                    for ik in range(KC):
                        nc.tensor.transpose(
                            pe[:, ik * 128:(ik + 1) * 128],
                            e_tile[:, ik * 128:(ik + 1) * 128], ident_b[:])
                    et_dst = (et_tiles[j][:] if o["fine_tiles"]
                              else et[:, :, j * 128:(j + 1) * 128])
                    if o["alt_copies"] and j % 2 == 1:
                        nc.vector.tensor_copy(
                            et_dst, pe[:].rearrange("p (a b) -> p a b", a=KC))
                    else:
                        copy_op(et_dst,
                                pe[:].rearrange("p (a b) -> p a b", a=KC))

                    for _b in range(o["bloat"]):
                        nc.vector.memset(scratch1[0:1, 0:1], 0.0)

                    if o["q2c_inline"]:
                        nc.scalar.activation(g[:, j:j + 1], mhat[:, j:j + 1],
                                             AF.Exp)
                        nc.tensor.matmul(psq2c[:], g[:, j:j + 1], cj[:],
                                         start=(j == 0), stop=(j == TT - 1),
                                         skip_group_check=True)
                        nc.tensor.matmul(psZ[:], g[:, j:j + 1], ones_col[:],
                                         start=(j == 0), stop=(j == TT - 1),
                                         skip_group_check=True)

                    # c block can go out as soon as loaded
                    if not o["early_cout"] and not o["o1_batch"]:
                        out_dma(j, slice(0, 512), c_f32(cj[:]))

                    if not o["two_pass"]:
                        do_mm2_epilogue(j, None)

                # ---------------- phase 2: q2c -------------------------------
                if not o["q2c_inline"]:
                    gsum = constp.tile([128, 1], f32, tag="gsum")
                    if o["g_accum"]:
                        nc.scalar.activation(g[:], mhat[:], AF.Exp,
                                             accum_out=gsum[:])
                    else:
                        nc.scalar.activation(g[:], mhat[:], AF.Exp)
                        nc.vector.reduce_sum(gsum[:], g[:], axis=AX.X)
                    psZ = ps_s.tile([1, 1], f32, tag="ps_s")
                    nc.tensor.matmul(psZ[:], ones_col[:], gsum[:],
                                     start=True, stop=True)
                    psq2c = ps_s.tile([1, D], f32, tag="ps_s")
                    for j in range(TT):
                        nc.tensor.matmul(psq2c[:], g[:, j:j + 1], c_tile(j),
                                         start=(j == 0), stop=(j == TT - 1))
                Zinv = constp.tile([1, 1], f32, tag="Zinv")
                nc.vector.reciprocal(Zinv[:], psZ[:])
                q2c_row = constp.tile([1, D], f32r if o["c_f32r"] else f32,
                                      tag="q2c_row")
                nc.vector.tensor_scalar_mul(q2c_row[:], psq2c[:], Zinv[:])

                psbc = ps_s.tile([128, D], f32, tag="ps_s")
                if o["c_f32r"]:
                    nc.tensor.matmul(psbc[:], ones_row[:], q2c_row[:],
                                     start=True, stop=True)
                else:
                    nc.tensor.matmul(psbc[:], ones_row_f[:], q2c_row[:],
                                     start=True, stop=True)
                q2c_bc = constp.tile([128, D], f32, tag="q2c_bc")
                copy_op(q2c_bc[:], psbc[:])

                # ---------------- phase 3 ------------------------------------
                if o["two_pass"]:
                    for j in range(TT):
                        do_mm2_epilogue(j, q2c_bc)
                else:
                    for j in range(TT):
                        jj, jr = divmod(j, 4)
                        if o["o4_split"]:
                            mul_e4 = nc.gpsimd if j % 2 else nc.vector
                        else:
                            mul_e4 = (nc.gpsimd if o["mul_eng"] == "gpsimd"
                                      else nc.vector)
                        o4 = outp.tile([128, D], f32, tag="o4")
                        mul_e4.tensor_mul(o4[:], c_f32(c_tile(j)[:]),
                                          q2c_bc[:])
                        out_dma(j, slice(1536, 2048), o4[:])

        if timing_mode:
            with tc.tile_pool(name="tickp", bufs=1) as tickp:
                tk = tickp.tile([1, 1], f32, tag="tick")
                nc.vector.memset(tk[:], 1.0)
                nc.sync.dma_start(tick_d[:], tk[:])

    nc.compile()
    return nc


# Default kernel: the v3 s-transposed formulation (sq via the matmul path;
# tensor_tensor_reduce and Act-engine f32r writes fault on HW), with input
# DMA dispatch split across SP/Act rings, setup loads + q_bf conversion on
# the otherwise-idle Pool engine, the phase-3 o4 muls weighted toward DVE
# (Pool muls are ~2.3x slower), and deep work/out pools.
KERNEL_OPTS = {"v3": True, "sq_mm": True, "in_alt": True,
               "bufs_work": 5, "bufs_out": 6, "setup_pool": True,
               "tail_dve": True}


def _get_built():
    global _BUILT
    if _BUILT is None:
        _BUILT = _build(opts=KERNEL_OPTS)
    return _BUILT


def kernel(c, q, w_c, b_c, w_q, b_q, w_cq, b_cq):
    """Full inputs in, full output out. Data-parallel over batch on 8 cores.

    Biases cancel mathematically (softmax shift invariance), so b_* are
    accepted but unused.
    """
    from concourse import bass_utils

    nc = _get_built()
    c = np.ascontiguousarray(np.asarray(c, dtype=np.float32))
    q = np.ascontiguousarray(np.asarray(q, dtype=np.float32))
    wc = np.ascontiguousarray(np.asarray(w_c, dtype=np.float32))
    wq = np.ascontiguousarray(np.asarray(w_q, dtype=np.float32))
    wcq = np.ascontiguousarray(np.asarray(w_cq, dtype=np.float32))

    in_maps = [
        {"c": c[b], "q": q[b], "wc": wc, "wq": wq, "wcq": wcq}
        for b in range(N_CORES)
    ]
    res = bass_utils.run_bass_kernel_spmd(
        nc, in_maps, core_ids=list(range(N_CORES)))
    return np.stack([res.results[b]["out"] for b in range(N_CORES)])



# revision 73
# speedup vs baseline: 1.8630x; 1.3181x over previous
"""Trainium2 Bass kernel for nn_AttentionFlow (BiDAF-style attention flow).

Math (per batch b, all biases cancel):
  s[t,i]   = <c_t,w_c> + <q_i,w_q> + <c_t*q_i, w_cq>  (+ biases)
  a        = softmax_i(s)          -> c2q = a @ q
  beta     = softmax_t(max_i s)    -> q2c = beta^T c
  out      = [c | c2q | c*c2q | c*q2c]

Key identities:
  * softmax_i(s[t,:]) is invariant to the per-row term sc[t] and all biases.
  * sc is folded into the matmul weights: qa[d,i] = q^T[d,i]*w_cq[d]+w_c[d].
  * exp(max_i s) = max_i exp(s), so beta's numerator comes from a max over
    the already-exponentiated E with no extra exp.
  * t and i orderings are arbitrary (softmax/sums are order-invariant and
    outputs are re-addressed by AP); i is stored as i = 4p + k.

Shipped kernel (_build_v3, KERNEL_OPTS): computes s TRANSPOSED per pair of
128-row tiles (256 t columns, which keeps f32r matmuls at full rate):
  s^T[i,t] = sum_d qa[d,i] C^T[d,t];  E^T = exp(s^T + sq[i]) via the Act
engine with sq as a per-partition bias (free); c2q and the row sums r come
from mm2 with E^T chunks stationary against q rows in natural layout plus a
ones column — no E transpose and no sq matmul. beta's g = max_i E^T via a
bf16 max tree + one 128-wide PE transpose. c/q live in SBUF as f32r
(DMA-bitcast) so every matmul and PE transpose runs at f32r rate; the
f32->f32r "rounded producer" rule is satisfied because DMA and DVE outputs
count as rounded (Act does NOT - it faults on HW - and tensor_tensor_reduce
faults outright).

Perf model (per core): 21MB HBM traffic (c 4MB in, q 1MB in, out 16MB) at
~360GB/s/core means a ~58us DMA roofline; engine busy (cost model) is
DVE 40us, PE 40us, Pool 32us, Act 31us, SP 27us - all below the roofline,
so the kernel is DMA-bound when the shared terminal is quiet and degrades
proportionally to HBM contention (cost-model cold critical path 74us, vs
121us for the session-start baseline). Output descriptors are
2KB+/partition, o1 (=c) goes out as one 4MB DMA, out-DMA dispatch
alternates SP/Pool rings, input dispatch alternates SP/Act rings, and
setup loads ride the otherwise-idle Pool ring, so no single sequencer
serializes the stream. The tail after the globally-dependent q2c is the
4MB o4 drain with its muls weighted onto DVE.

Sharding: data-parallel over batch, one batch element per NeuronCore (8).
"""

import numpy as np

N_CORES = 8
T, I, D = 2048, 512, 512
TT = T // 128  # 16 row tiles
KC = 4         # 128-chunks of D (and of I)

DEFAULT_OPTS = dict(contig_in=True, out_ring="sync", two_pass=False,
                    skip_out=False, memset_in=False, dma_pair=False,
                    bufs_work=3, bufs_out=3, out_split=False,
                    dup_pe=False, dup_dve=False, dup_act=False, bloat=0,
                    act_copies="act", exp_accum=True, g_accum=True,
                    mul_eng="gpsimd", batch_recip=False, alt_copies=False,
                    split_in=True, q2c_inline=False, o4_split=True,
                    ps_tr_bufs=2, ct_eng="dve", early_cout=False,
                    ps_s_bufs=2, ps_mm2_bufs=2, fine_tiles=True, fine_c=False,
                    c_onebuf=True, o1_batch=True, tr_f32r=False,
                    q2c_f32r=False, out_alt=True, c_f32r=True)

_BUILT = None


def _build_v3(reps=1, timing_mode=False, opts=None):
    """s-transposed formulation.

    Per pair of 128-row tiles (256 t-columns, satisfying the f32r
    ap>=256 full-rate rule):
      s^T[i,t] = sum_d qa[d,i] * C^T[d,t]   (qa = Q^T*wcq + wc, folds sc)
      E^T = exp(s^T + sq[i])                (sq per-partition Act bias)
      g[t] = max_i E^T[i,t]  (= exp(max_i s) by monotonicity; bf16 max
             tree + one 128-wide PE transpose + free-axis reduce_max)
      c2q-row r[t] and c2q via mm2 with E^T chunks as stationary weights
             (rhs = q rows natural layout; ones column gives r).
    No E transpose, no sq matmul, q2c contraction in f32r.
    """
    import concourse.tile as tile
    from concourse import bacc, mybir
    from concourse.masks import make_identity

    o = dict(v3_mul_split=True, out_alt=True, o1_batch=True,
             seq_pr=False, sq_mm=False, in_alt=False, bufs_work=3,
             bufs_out=3, ct_alt=False, setup_pool=False, tail_dve=False,
             q2c_il=False, o1_split=1, q_chunked=False, pr_pack=False)
    if opts:
        o.update(opts)

    f32 = mybir.dt.float32
    f32r = mybir.dt.float32r
    bf16 = mybir.dt.bfloat16
    AF = mybir.ActivationFunctionType
    AX = mybir.AxisListType
    ALU = mybir.AluOpType
    IC = 4

    nc = bacc.Bacc("TRN2", target_bir_lowering=False, debug=False,
                   num_devices=N_CORES)
    c_d = nc.dram_tensor("c", [T, D], f32, kind="ExternalInput").ap()
    q_d = nc.dram_tensor("q", [I, D], f32, kind="ExternalInput").ap()
    wc_d = nc.dram_tensor("wc", [D], f32, kind="ExternalInput").ap()
    wq_d = nc.dram_tensor("wq", [D], f32, kind="ExternalInput").ap()
    wcq_d = nc.dram_tensor("wcq", [D], f32, kind="ExternalInput").ap()
    out_kind = "Internal" if timing_mode else "ExternalOutput"
    out_d = nc.dram_tensor("out", [T, 4 * D], f32, kind=out_kind).ap()
    tick_d = (nc.dram_tensor("tick", [1, 1], f32, kind="ExternalOutput").ap()
              if timing_mode else None)

    with tile.TileContext(nc) as tc:
        with (
            tc.tile_pool(name="const", bufs=1) as constp,
            tc.tile_pool(name="big", bufs=1) as bigp,
            tc.tile_pool(name="work", bufs=o["bufs_work"]) as workp,
            tc.tile_pool(name="outp", bufs=o["bufs_out"]) as outp,
            tc.tile_pool(name="ps_tr", bufs=2, space="PSUM") as ps_tr,
            tc.tile_pool(name="ps_s",
                         bufs=2 if (o["q2c_il"] and not o["pr_pack"]) else 3,
                         space="PSUM") as ps_s,
            tc.tile_pool(name="ps_mm2", bufs=2, space="PSUM") as ps_mm2,
            tc.tile_pool(name="ps_sm", bufs=1, space="PSUM") as ps_sm,
            tc.tile_pool(name="ps_q2i", bufs=1, space="PSUM") as ps_q2i,
        ):
            for _rep in range(reps):
                # ---------------- setup --------------------------------------
                ident_f = constp.tile([128, 128], f32, tag="idf")
                make_identity(nc, ident_f[:])
                ident_b = constp.tile([128, 128], bf16, tag="idb")
                make_identity(nc, ident_b[:])
                ident_r = constp.tile([128, 128], f32r, tag="idr")
                nc.vector.tensor_copy(ident_r[:], ident_f[:])
                ones_row_f = constp.tile([1, 128], f32, tag="ones_row_f")
                nc.vector.memset(ones_row_f[:], 1.0)
                ones_row = constp.tile([1, 128], f32r, tag="ones_row")
                nc.vector.tensor_copy(ones_row[:], ones_row_f[:])
                ones_col = constp.tile([128, 1], f32, tag="ones_col")
                nc.vector.memset(ones_col[:], 1.0)
                ones_col_b = constp.tile([128, 1], bf16, tag="ones_col_b")
                nc.vector.memset(ones_col_b[:], 1.0)

                setup_eng = nc.gpsimd if o["setup_pool"] else nc.sync
                wcq_col = constp.tile([128, KC], f32, tag="wcq_col")
                setup_eng.dma_start(wcq_col[:],
                                    wcq_d.rearrange("(a b) -> b a", b=128))
                wc_col = constp.tile([128, KC], f32, tag="wc_col")
                setup_eng.dma_start(wc_col[:],
                                    wc_d.rearrange("(a b) -> b a", b=128))
                wq_row = constp.tile([1, D], f32, tag="wq_row")
                setup_eng.dma_start(wq_row[:],
                                    wq_d.rearrange("(a d) -> a d", a=1))

                # q in [i_part, d] with i = 4p+k; f32r so PE transposes and
                # setup matmuls run at full f32r rate
                q_sb = bigp.tile([128, KC, D], f32r, tag="q_sb")
                qrs = q_d.rearrange("(p k) d -> p k d", k=KC).bitcast(f32r)
                if o["q_chunked"]:
                    for _k in range(KC):
                        setup_eng.dma_start(q_sb[:, _k, :], qrs[:, _k, :])
                else:
                    setup_eng.dma_start(q_sb[:], qrs)
                q_bf = bigp.tile([128, KC, D], bf16, tag="q_bf")
                if o["setup_pool"]:
                    nc.gpsimd.tensor_copy(q_bf[:], q_sb[:].bitcast(f32))
                else:
                    nc.vector.tensor_copy(q_bf[:], q_sb[:].bitcast(f32))

                # c in f32r, one contiguous buffer, fine-grained loads
                crs = c_d.rearrange("(p j) d -> p j d", j=TT)
                c_big = bigp.tile([128, TT, D], f32r, tag="c_big")
                for _j in range(TT):
                    in_eng = (nc.scalar if (o["in_alt"] and _j % 2)
                              else nc.sync)
                    in_eng.dma_start(c_big[:, _j, :],
                                     crs[:, _j, :].bitcast(f32r))

                ors = out_d.rearrange("(p j) w -> p j w", j=TT)

                _out_n = [0]

                def out_dma(j, sl, src):
                    _out_n[0] += 1
                    eng = (nc.gpsimd if (o["out_alt"] and _out_n[0] % 2)
                           else nc.sync)
                    eng.dma_start(ors[:, j, sl], src)

                # o1 = c passthrough, one (or a few) big DMAs
                if o["o1_batch"]:
                    nsp = o["o1_split"]
                    w = TT // nsp
                    for sp in range(nsp):
                        nc.sync.dma_start(
                            ors[:, sp * w:(sp + 1) * w, 0:512],
                            c_big[:, sp * w:(sp + 1) * w, :].bitcast(f32))

                sq_col = constp.tile([128, KC], f32, tag="sq_col")
                if not o["sq_mm"]:
                    # wq broadcast to all partitions (via PE), then per-row
                    # dots: sq_col[p, k] = <q[4p+k, :], wq>
                    ps_bc = ps_mm2.tile([128, D], f32, tag="pc")
                    nc.tensor.matmul(ps_bc[:], ones_row_f[:], wq_row[:],
                                     start=True, stop=True)
                    wq_bc = constp.tile([128, D], f32, tag="wq_bc")
                    nc.scalar.copy(wq_bc[:], ps_bc[:])
                    sq_scr = constp.tile([128, D], f32, tag="sq_scr")
                    for k in range(KC):
                        nc.vector.tensor_tensor_reduce(
                            sq_scr[:], q_sb[:, k], wq_bc[:], 1.0, 0.0,
                            ALU.mult, ALU.add, accum_out=sq_col[:, k:k + 1])

                # qa[d, i] = Q^T * wcq + wc  (via PE transposes of q)
                qa = bigp.tile([128, KC, I], f32r, tag="qa")
                if o["sq_mm"]:
                    qt = bigp.tile([128, KC, I], f32r, tag="qt")
                else:
                    qt = None
                for k in range(KC):
                    pt = ps_mm2.tile([128, I], f32, tag="pc")
                    for ik in range(KC):
                        nc.tensor.transpose(
                            pt[:, ik * 128:(ik + 1) * 128].bitcast(f32r),
                            q_sb[:, ik, k * 128:(k + 1) * 128],
                            ident_r[:])
                    if o["sq_mm"]:
                        nc.vector.tensor_copy(qt[:, k], pt[:])
                    nc.vector.tensor_scalar(
                        qa[:, k], pt[:], wcq_col[:, k:k + 1],
                        wc_col[:, k:k + 1], op0=ALU.mult, op1=ALU.add)

                if o["sq_mm"]:
                    # sq_row = wq^T Q^T, then 4 thin transposes into sq_col
                    wq_col = constp.tile([128, KC], f32r, tag="wq_col")
                    nc.sync.dma_start(
                        wq_col[:],
                        wq_d.rearrange("(a b) -> b a", b=128).bitcast(f32r))
                    ps_sq = ps_mm2.tile([1, I], f32, tag="pc")
                    for k in range(KC):
                        nc.tensor.matmul(ps_sq[:], wq_col[:, k:k + 1],
                                         qt[:, k], start=(k == 0),
                                         stop=(k == KC - 1))
                    sq_row = constp.tile([1, I], f32, tag="sq_row")
                    nc.scalar.copy(sq_row[:], ps_sq[:])
                    sq_pool, sq_tag = ((ps_s, "psT") if o["pr_pack"]
                                       else (ps_sm, "pr"))
                    sq_ps = sq_pool.tile([128, KC], f32, tag=sq_tag)
                    for ic in range(IC):
                        nc.tensor.transpose(
                            sq_ps[:, ic:ic + 1],
                            sq_row[0:1, ic * 128:(ic + 1) * 128],
                            ones_row_f[0:1, 0:1])
                    nc.vector.tensor_copy(sq_col[:], sq_ps[:])

                g = constp.tile([128, TT], f32r, tag="g")
                ri_tiles = []
                for _j in range(TT):
                    ri_j = bigp.tile([128, 1], f32, tag=f"ri{_j}")
                    ri_tiles.append(ri_j)

                if o["q2c_il"]:
                    psq2c_il = ps_q2i.tile([1, D], f32, tag="q2i")

                # ---------------- phase 1: per pair of tiles -----------------
                for m in range(TT // 2):
                    j0 = 2 * m

                    # C^T for the pair: [d_part, 256 t]
                    ct = workp.tile([128, KC, 256], f32r, tag="ct")
                    for k in range(KC):
                        ptk = ps_tr.tile([128, 256], f32, tag="ptk")
                        for jj in range(2):
                            nc.tensor.transpose(
                                ptk[:, jj * 128:(jj + 1) * 128]
                                .bitcast(f32r),
                                c_big[:, j0 + jj,
                                      k * 128:(k + 1) * 128],
                                ident_r[:])
                        if o["ct_alt"] and k % 2:
                            nc.gpsimd.tensor_copy(ct[:, k], ptk[:])
                        else:
                            nc.vector.tensor_copy(ct[:, k], ptk[:])

                    # mm1 per i-chunk + exp + bf16 max tree
                    et2 = workp.tile([128, IC, 256], bf16, tag="et2")
                    m4e = workp.tile([128, 256], bf16, tag="m4e")
                    for ic in range(IC):
                        psT = ps_s.tile([128, 256], f32, tag="psT")
                        for k in range(KC):
                            nc.tensor.matmul(
                                psT[:], qa[:, k, ic * 128:(ic + 1) * 128],
                                ct[:, k], start=(k == 0), stop=(k == KC - 1))
                        nc.scalar.activation(et2[:, ic, :], psT[:], AF.Exp,
                                             bias=sq_col[:, ic:ic + 1])
                        if ic == 0:
                            nc.vector.tensor_copy(m4e[:], et2[:, 0, :])
                        else:
                            nc.vector.tensor_tensor(
                                m4e[:], m4e[:], et2[:, ic, :], ALU.max)

                    # per tile: g column, mm2, epilogue
                    for jj in range(2):
                        j = j0 + jj
                        mt = ps_tr.tile([128, 128], bf16, tag="ptk")
                        nc.tensor.transpose(
                            mt[:], m4e[:, jj * 128:(jj + 1) * 128],
                            ident_b[:])
                        nc.vector.reduce_max(g[:, j:j + 1], mt[:], axis=AX.X)
                        if o["q2c_il"]:
                            nc.tensor.matmul(psq2c_il[:], g[:, j:j + 1],
                                             c_big[:, j], start=(j == 0),
                                             stop=(j == TT - 1),
                                             skip_group_check=True)

                        pc = ps_mm2.tile([128, 512], f32, tag="pc")
                        if o["pr_pack"]:
                            pr = ps_s.tile([128, 1], f32, tag="psT")
                        else:
                            pr = ps_sm.tile([128, 1], f32, tag="pr")
                        if o["seq_pr"]:
                            for ic in range(IC):
                                nc.tensor.matmul(
                                    pc[:], et2[:, ic, jj * 128:(jj + 1) * 128],
                                    q_bf[:, ic], start=(ic == 0),
                                    stop=(ic == IC - 1))
                            for ic in range(IC):
                                nc.tensor.matmul(
                                    pr[:], et2[:, ic, jj * 128:(jj + 1) * 128],
                                    ones_col_b[:], start=(ic == 0),
                                    stop=(ic == IC - 1))
                        else:
                            for ic in range(IC):
                                lhs = et2[:, ic, jj * 128:(jj + 1) * 128]
                                nc.tensor.matmul(pc[:], lhs, q_bf[:, ic],
                                                 start=(ic == 0),
                                                 stop=(ic == IC - 1))
                                nc.tensor.matmul(pr[:], lhs, ones_col_b[:],
                                                 start=(ic == 0),
                                                 stop=(ic == IC - 1),
                                                 skip_group_check=True)
                        nc.vector.reciprocal(ri_tiles[j][:], pr[:])
                        o_t = outp.tile([128, 1024], f32, tag="o23")
                        nc.scalar.mul(o_t[:, 0:512], pc[:], ri_tiles[j][:])
                        mul_e = (nc.gpsimd if (o["v3_mul_split"] and j % 2)
                                 else nc.vector)
                        mul_e.tensor_mul(o_t[:, 512:1024],
                                         c_big[:, j].bitcast(f32),
                                         o_t[:, 0:512])
                        out_dma(j, slice(512, 1536), o_t[:])

                # ---------------- phase 2: q2c -------------------------------
                gsum = constp.tile([128, 1], f32, tag="gsum")
                nc.vector.reduce_sum(gsum[:], g[:], axis=AX.X)
                if o["pr_pack"]:
                    psZ = ps_s.tile([1, 1], f32, tag="psT")
                else:
                    psZ = ps_sm.tile([1, 1], f32, tag="pr")
                nc.tensor.matmul(psZ[:], ones_col[:], gsum[:],
                                 start=True, stop=True)
                if o["q2c_il"]:
                    psq2c = psq2c_il
                else:
                    psq2c = ps_mm2.tile([1, D], f32, tag="pc")
                    for j in range(TT):
                        nc.tensor.matmul(psq2c[:], g[:, j:j + 1],
                                         c_big[:, j], start=(j == 0),
                                         stop=(j == TT - 1))
                Zinv = constp.tile([1, 1], f32, tag="Zinv")
                nc.vector.reciprocal(Zinv[:], psZ[:])
                q2c_row = constp.tile([1, D], f32r, tag="q2c_row")
                nc.vector.tensor_scalar_mul(q2c_row[:], psq2c[:], Zinv[:])
                psbc = ps_mm2.tile([128, D], f32, tag="pc")
                nc.tensor.matmul(psbc[:], ones_row[:], q2c_row[:],
                                 start=True, stop=True)
                q2c_bc = constp.tile([128, D], f32, tag="q2c_bc")
                nc.scalar.copy(q2c_bc[:], psbc[:])

                # ---------------- phase 3: o4 --------------------------------
                for j in range(TT):
                    if o["tail_dve"]:
                        # Pool muls are ~2.3x slower; weight toward DVE
                        mul_e4 = nc.gpsimd if j % 3 == 2 else nc.vector
                    else:
                        mul_e4 = nc.gpsimd if j % 2 else nc.vector
                    o4 = outp.tile([128, D], f32, tag="o4")
                    mul_e4.tensor_mul(o4[:], c_big[:, j].bitcast(f32),
                                      q2c_bc[:])
                    out_dma(j, slice(1536, 2048), o4[:])

        if timing_mode:
            with tc.tile_pool(name="tickp", bufs=1) as tickp:
                tk = tickp.tile([1, 1], f32, tag="tick")
                nc.vector.memset(tk[:], 1.0)
                nc.sync.dma_start(tick_d[:], tk[:])

    nc.compile()
    return nc


def _build(reps=1, timing_mode=False, opts=None):
    if opts and opts.get("v3"):
        o2 = {k: v for k, v in opts.items() if k != "v3"}
        return _build_v3(reps, timing_mode, o2)
    import concourse.tile as tile
    from concourse import bacc, mybir
    from concourse.masks import make_identity

    o = dict(DEFAULT_OPTS)
    if opts:
        o.update(opts)

    f32 = mybir.dt.float32
    f32r = mybir.dt.float32r
    bf16 = mybir.dt.bfloat16
    AF = mybir.ActivationFunctionType
    AX = mybir.AxisListType
    ALU = mybir.AluOpType

    nc = bacc.Bacc("TRN2", target_bir_lowering=False, debug=False,
                   num_devices=N_CORES)
    c_d = nc.dram_tensor("c", [T, D], f32, kind="ExternalInput").ap()
    q_d = nc.dram_tensor("q", [I, D], f32, kind="ExternalInput").ap()
    wc_d = nc.dram_tensor("wc", [D], f32, kind="ExternalInput").ap()
    wq_d = nc.dram_tensor("wq", [D], f32, kind="ExternalInput").ap()
    wcq_d = nc.dram_tensor("wcq", [D], f32, kind="ExternalInput").ap()
    out_kind = "Internal" if timing_mode else "ExternalOutput"
    out_d = nc.dram_tensor("out", [T, 4 * D], f32, kind=out_kind).ap()
    tick_d = (nc.dram_tensor("tick", [1, 1], f32, kind="ExternalOutput").ap()
              if timing_mode else None)

    out_eng = {"sync": nc.sync, "scalar": nc.scalar, "gpsimd": nc.gpsimd,
               "vector": nc.vector}[o["out_ring"]]

    with tile.TileContext(nc) as tc:
        with (
            tc.tile_pool(name="const", bufs=1) as constp,
            tc.tile_pool(name="big", bufs=1) as bigp,
            tc.tile_pool(name="work", bufs=o["bufs_work"]) as workp,
            tc.tile_pool(name="outp", bufs=o["bufs_out"]) as outp,
            tc.tile_pool(name="ps_tr", bufs=o["ps_tr_bufs"],
                         space="PSUM") as ps_tr,
            tc.tile_pool(name="ps_acc", bufs=1, space="PSUM") as ps_acc,
            tc.tile_pool(name="ps_s", bufs=o["ps_s_bufs"],
                         space="PSUM") as ps_s,
            tc.tile_pool(name="ps_mm2", bufs=o["ps_mm2_bufs"],
                         space="PSUM") as ps_mm2,
        ):
            for _rep in range(reps):
                # ---------------- phase 0 -----------------------------------
                ident_f = constp.tile([128, 128], f32, tag="idf")
                make_identity(nc, ident_f[:])
                ident_b = constp.tile([128, 128], bf16, tag="idb")
                make_identity(nc, ident_b[:])

                if o["c_f32r"]:
                    ident_rt = constp.tile([128, 128], f32r, tag="idr")
                    nc.vector.tensor_copy(ident_rt[:], ident_f[:])
                    ident_r = ident_rt[:]
                ones_row_f = constp.tile([1, 128], f32, tag="ones_row_f")
                nc.vector.memset(ones_row_f[:], 1.0)
                ones_row = constp.tile([1, 128], f32r, tag="ones_row")
                nc.vector.tensor_copy(ones_row[:], ones_row_f[:])
                ones_col = constp.tile([128, 1], f32, tag="ones_col")
                nc.vector.memset(ones_col[:], 1.0)

                wcq_col = constp.tile([128, KC], f32, tag="wcq_col")
                nc.sync.dma_start(wcq_col[:],
                                  wcq_d.rearrange("(a b) -> b a", b=128))
                wc_col = constp.tile([128, KC], f32, tag="wc_col")
                nc.sync.dma_start(wc_col[:],
                                  wc_d.rearrange("(a b) -> b a", b=128))
                wq_col = constp.tile([128, KC], f32, tag="wq_col")
                nc.sync.dma_start(wq_col[:],
                                  wq_d.rearrange("(a b) -> b a", b=128))

                q_sb = bigp.tile([128, KC, D], f32, tag="q_sb")
                if o["memset_in"]:
                    nc.gpsimd.memset(q_sb[:], 0.01)
                elif o["contig_in"]:
                    nc.sync.dma_start(
                        q_sb[:], q_d.rearrange("(p k) d -> p k d", k=KC))
                else:
                    nc.sync.dma_start(
                        q_sb[:], q_d.rearrange("(k p) d -> p k d", p=128))
                q_bf = bigp.tile([128, KC, D], bf16, tag="q_bf")
                nc.vector.tensor_copy(q_bf[:], q_sb[:])

                c_sb = []
                if o["memset_in"]:
                    for jj in range(4):
                        t_ = bigp.tile([128, 4, D], f32, tag=f"c_sb{jj}")
                        nc.gpsimd.memset(t_[:], 0.02)
                        c_sb.append(t_)
                elif o["contig_in"] and o["c_onebuf"]:
                    crs = c_d.rearrange("(p j) d -> p j d", j=TT)
                    c_dt = f32r if o["c_f32r"] else f32
                    c_big = bigp.tile([128, TT, D], c_dt, tag="c_big")
                    for _j in range(TT):
                        if o["c_f32r"]:
                            nc.sync.dma_start(c_big[:, _j, :],
                                              crs[:, _j, :].bitcast(f32r))
                        else:
                            nc.sync.dma_start(c_big[:, _j, :], crs[:, _j, :])
                elif o["contig_in"] and o["fine_c"]:
                    crs = c_d.rearrange("(p j) d -> p j d", j=TT)
                    c_fine = []
                    for _j in range(TT):
                        cf = bigp.tile([128, D], f32, tag=f"cin{_j}")
                        nc.sync.dma_start(cf[:], crs[:, _j, :])
                        c_fine.append(cf)
                elif o["contig_in"]:
                    crs = c_d.rearrange("(p j) d -> p j d", j=TT)
                    if o["split_in"]:
                        for jj in range(4):
                            t_ = bigp.tile([128, 4, D], f32, tag=f"c_sb{jj}")
                            for jr in range(4):
                                nc.sync.dma_start(
                                    t_[:, jr:jr + 1, :],
                                    crs[:, 4 * jj + jr:4 * jj + jr + 1, :])
                            c_sb.append(t_)
                    else:
                        for jj in range(4):
                            t_ = bigp.tile([128, 4, D], f32, tag=f"c_sb{jj}")
                            nc.sync.dma_start(t_[:],
                                              crs[:, 4 * jj:4 * jj + 4, :])
                            c_sb.append(t_)
                else:
                    for jj in range(4):
                        t_ = bigp.tile([128, 4, D], f32, tag=f"c_sb{jj}")
                        nc.sync.dma_start(
                            t_[:],
                            c_d[jj * 512:(jj + 1) * 512, :].rearrange(
                                "(j p) d -> p j d", p=128))
                        c_sb.append(t_)

                if o["contig_in"]:
                    ors = out_d.rearrange("(p j) w -> p j w", j=TT)

                    def out_ap(j, sl):
                        return ors[:, j, sl]
                else:
                    def out_ap(j, sl):
                        return out_d[j * 128:(j + 1) * 128, sl]

                if o["c_f32r"]:
                    assert o["c_onebuf"], "c_f32r requires c_onebuf"

                def c_tile(j):
                    if o["contig_in"] and o["c_onebuf"]:
                        return c_big[:, j]
                    if o["contig_in"] and o["fine_c"]:
                        return c_fine[j]
                    jj_, jr_ = divmod(j, 4)
                    return c_sb[jj_][:, jr_]

                def c_f32(ap):
                    # f32 view of c for DVE/Pool/DMA when stored as f32r
                    return ap.bitcast(f32) if o["c_f32r"] else ap

                _out_n = [0]

                def out_dma(j, sl, src):
                    if o["skip_out"]:
                        return
                    _out_n[0] += 1
                    if o["out_alt"]:
                        eng = nc.gpsimd if _out_n[0] % 2 else nc.sync
                    elif o["out_split"] and _out_n[0] % 2:
                        eng = nc.scalar
                    else:
                        eng = out_eng
                    eng.dma_start(out_ap(j, sl), src)

                if o["dma_pair"]:
                    for j in range(TT):
                        jj, jr = divmod(j, 4)
                        cj = c_sb[jj][:, jr]
                        out_dma(j, slice(0, 512), cj[:])
                        out_dma(j, slice(512, 2048),
                                c_sb[jj][:].rearrange("p a d -> p (a d)")
                                [:, 0:1536])
                    continue

                def copy_op(dst, src):
                    if o["act_copies"] == "dve":
                        nc.vector.tensor_copy(dst, src)
                    else:
                        nc.scalar.copy(dst, src)

                if o["o1_batch"]:
                    if not o["skip_out"]:
                        out_eng.dma_start(ors[:, :, 0:512], c_f32(c_big[:]))
                elif o["early_cout"] and not o["dma_pair"]:
                    for j in range(TT):
                        out_dma(j, slice(0, 512), c_tile(j))

                # Q^T, qa = Q^T * wcq + wc
                qt = bigp.tile([128, KC, I], f32, tag="qt")
                qa = bigp.tile([128, KC, I], f32r, tag="qa")
                for k in range(KC):
                    pt = ps_tr.tile([128, I], f32, tag="ps_tr")
                    for ik in range(KC):
                        nc.tensor.transpose(
                            pt[:, ik * 128:(ik + 1) * 128],
                            q_sb[:, ik, k * 128:(k + 1) * 128],
                            ident_f[:])
                    copy_op(qt[:, k], pt[:])
                    nc.vector.tensor_scalar(
                        qa[:, k], pt[:], wcq_col[:, k:k + 1],
                        wc_col[:, k:k + 1], op0=ALU.mult, op1=ALU.add)

                # sq_row[1, I] = w_q^T Q^T
                ps_sq = ps_s.tile([1, I], f32, tag="ps_s")
                for k in range(KC):
                    nc.tensor.matmul(ps_sq[:], wq_col[:, k:k + 1], qt[:, k],
                                     start=(k == 0), stop=(k == KC - 1))
                sq_row = constp.tile([1, I], f32r, tag="sq_row")
                copy_op(sq_row[:], ps_sq[:])

                scratch1 = constp.tile([1, 1], f32, tag="scratch1")
                g = constp.tile([128, TT], f32r if o["c_f32r"] else f32,
                                tag="g")
                if o["q2c_inline"]:
                    psq2c = ps_acc.tile([1, D], f32, tag="ps_q2c")
                    psZ = ps_acc.tile([1, 1], f32, tag="ps_Z")
                mhat = constp.tile([128, TT], f32, tag="mhat")
                r_col = constp.tile([128, TT], f32, tag="r_col")
                rinv = constp.tile([128, TT], f32, tag="rinv")
                if o["fine_tiles"]:
                    et_tiles = []
                    for _j in range(TT):
                        et_j = bigp.tile([128, KC, 128], bf16,
                                         tag=f"et{_j}")
                        et_tiles.append(et_j)
                    r_tiles = []
                    ri_tiles = []
                    for _j in range(TT):
                        r_j = bigp.tile([128, 1], f32, tag=f"r{_j}")
                        r_tiles.append(r_j)
                        ri_j = bigp.tile([128, 1], f32, tag=f"ri{_j}")
                        ri_tiles.append(ri_j)
                else:
                    et = bigp.tile([128, KC, T], bf16, tag="et")

                # ---------------- phase 1: per row-tile ----------------------
                def do_mm2_epilogue(j, q2c_bc):
                    cj = c_tile(j)
                    pc = ps_mm2.tile([128, D], f32, tag="ps_mm2")
                    for ik in range(KC):
                        lhs_mm2 = (et_tiles[j][:, ik, :] if o["fine_tiles"]
                                   else et[:, ik, j * 128:(j + 1) * 128])
                        nc.tensor.matmul(pc[:], lhs_mm2, q_bf[:, ik],
                                         start=(ik == 0), stop=(ik == KC - 1))
                    if q2c_bc is None:
                        o_t = outp.tile([128, 1024], f32, tag="o23")
                        if o["act_copies"] == "dve":
                            nc.vector.tensor_scalar_mul(o_t[:, 0:512], pc[:],
                                                        (ri_tiles[j][:] if o["fine_tiles"] else rinv[:, j:j + 1]))
                        else:
                            nc.scalar.mul(o_t[:, 0:512], pc[:],
                                          (ri_tiles[j][:] if o["fine_tiles"]
                                           else rinv[:, j:j + 1]))
                        mul_e = (nc.gpsimd if o["mul_eng"] == "gpsimd"
                                 else nc.vector)
                        mul_e.tensor_mul(o_t[:, 512:1024], c_f32(cj[:]),
                                         o_t[:, 0:512])
                        if o["dup_dve"]:
                            nc.vector.tensor_mul(o_t[:, 512:1024],
                                                 c_f32(cj[:]),
                                                 o_t[:, 0:512])
                        out_dma(j, slice(512, 1536), o_t[:])
                    else:
                        o_t = outp.tile([128, 1536], f32, tag="o234")
                        if o["act_copies"] == "dve":
                            nc.vector.tensor_scalar_mul(o_t[:, 0:512], pc[:],
                                                        (ri_tiles[j][:] if o["fine_tiles"] else rinv[:, j:j + 1]))
                        else:
                            nc.scalar.mul(o_t[:, 0:512], pc[:],
                                          (ri_tiles[j][:] if o["fine_tiles"]
                                           else rinv[:, j:j + 1]))
                        nc.vector.tensor_mul(o_t[:, 512:1024], c_f32(cj[:]),
                                             o_t[:, 0:512])
                        nc.vector.tensor_mul(o_t[:, 1024:1536], c_f32(cj[:]),
                                             q2c_bc[:])
                        out_dma(j, slice(512, 2048), o_t[:])

                for j in range(TT):
                    cj = c_tile(j)  # [128, 512] fp32

                    # C^T for this tile
                    pt = ps_tr.tile([128, 512], f32, tag="ps_tr")
                    if o["c_f32r"]:
                        for k in range(KC):
                            nc.tensor.transpose(
                                pt[:, k * 128:(k + 1) * 128].bitcast(f32r),
                                cj[:, k * 128:(k + 1) * 128], ident_r)
                    else:
                        for k in range(KC):
                            nc.tensor.transpose(
                                pt[:, k * 128:(k + 1) * 128],
                                cj[:, k * 128:(k + 1) * 128], ident_f[:])
                    ct = workp.tile([128, 512], f32r, tag="ct")
                    if o["ct_eng"] == "act" or (o["alt_copies"] and j % 2 == 0):
                        nc.scalar.copy(ct[:], pt[:])
                    else:
                        nc.vector.tensor_copy(ct[:], pt[:])
                    if o["dup_dve"]:
                        nc.vector.tensor_copy(ct[:], pt[:])

                    # mm1: s' = c @ qa + 1*sq
                    ps = ps_s.tile([128, I], f32, tag="ps_s")
                    if o["dup_pe"]:
                        for k in range(KC):
                            nc.tensor.matmul(
                                ps[:], ct[:, k * 128:(k + 1) * 128],
                                qa[:, k], start=(k == 0), stop=False,
                                skip_group_check=True)
                        for k in range(KC):
                            nc.tensor.matmul(
                                ps[:], ct[:, k * 128:(k + 1) * 128],
                                qa[:, k], start=(k == 0), stop=False,
                                skip_group_check=True)
                    else:
                        for k in range(KC):
                            nc.tensor.matmul(
                                ps[:], ct[:, k * 128:(k + 1) * 128],
                                qa[:, k], start=(k == 0), stop=False)
                    nc.tensor.matmul(ps[:], ones_row[:], sq_row[:],
                                     start=False, stop=True)

                    nc.vector.reduce_max(mhat[:, j:j + 1], ps[:], axis=AX.X)

                    e_tile = workp.tile([128, I], bf16, tag="e")
                    r_dst = (r_tiles[j][:] if o["fine_tiles"]
                             else r_col[:, j:j + 1])
                    if o["exp_accum"]:
                        nc.scalar.activation(e_tile[:], ps[:], AF.Exp,
                                             accum_out=r_dst)
                    else:
                        nc.scalar.activation(e_tile[:], ps[:], AF.Exp)
                        nc.vector.reduce_sum(r_dst, e_tile[:], axis=AX.X)
                    if o["dup_act"]:
                        nc.scalar.activation(e_tile[:], ps[:], AF.Exp,
                                             accum_out=r_col[:, j:j + 1])
                    if o["fine_tiles"]:
                        nc.vector.reciprocal(ri_tiles[j][:], r_tiles[j][:])
                    elif o["batch_recip"]:
                        if j % 4 == 3:
                            nc.vector.reciprocal(rinv[:, j - 3:j + 1],
                                                 r_col[:, j - 3:j + 1])
                    else:
                        nc.vector.reciprocal(rinv[:, j:j + 1],
                                             r_col[:, j:j + 1])

                    # E^T into et[:, ik, j*128:...]
                    pe = ps_tr.tile([128, 512], bf16, tag="ps_tr")
                    for ik in range(KC):
                        nc.tensor.transpose(
                            pe[:, ik * 128:(ik + 1) * 128],
                            e_tile[:, ik * 128:(ik + 1) * 128], ident_b[:])
                    et_dst = (et_tiles[j][:] if o["fine_tiles"]
                              else et[:, :, j * 128:(j + 1) * 128])
                    if o["alt_copies"] and j % 2 == 1:
                        nc.vector.tensor_copy(
                            et_dst, pe[:].rearrange("p (a b) -> p a b", a=KC))
                    else:
                        copy_op(et_dst,
                                pe[:].rearrange("p (a b) -> p a b", a=KC))

                    for _b in range(o["bloat"]):
                        nc.vector.memset(scratch1[0:1, 0:1], 0.0)

                    if o["q2c_inline"]:
                        nc.scalar.activation(g[:, j:j + 1], mhat[:, j:j + 1],
                                             AF.Exp)
                        nc.tensor.matmul(psq2c[:], g[:, j:j + 1], cj[:],
                                         start=(j == 0), stop=(j == TT - 1),
                                         skip_group_check=True)
                        nc.tensor.matmul(psZ[:], g[:, j:j + 1], ones_col[:],
                                         start=(j == 0), stop=(j == TT - 1),
                                         skip_group_check=True)

                    # c block can go out as soon as loaded
                    if not o["early_cout"] and not o["o1_batch"]:
                        out_dma(j, slice(0, 512), c_f32(cj[:]))

                    if not o["two_pass"]:
                        do_mm2_epilogue(j, None)

                # ---------------- phase 2: q2c -------------------------------
                if not o["q2c_inline"]:
                    gsum = constp.tile([128, 1], f32, tag="gsum")
                    if o["g_accum"]:
                        nc.scalar.activation(g[:], mhat[:], AF.Exp,
                                             accum_out=gsum[:])
                    else:
                        nc.scalar.activation(g[:], mhat[:], AF.Exp)
                        nc.vector.reduce_sum(gsum[:], g[:], axis=AX.X)
                    psZ = ps_s.tile([1, 1], f32, tag="ps_s")
                    nc.tensor.matmul(psZ[:], ones_col[:], gsum[:],
                                     start=True, stop=True)
                    psq2c = ps_s.tile([1, D], f32, tag="ps_s")
                    for j in range(TT):
                        nc.tensor.matmul(psq2c[:], g[:, j:j + 1], c_tile(j),
                                         start=(j == 0), stop=(j == TT - 1))
                Zinv = constp.tile([1, 1], f32, tag="Zinv")
                nc.vector.reciprocal(Zinv[:], psZ[:])
                q2c_row = constp.tile([1, D], f32r if o["c_f32r"] else f32,
                                      tag="q2c_row")
                nc.vector.tensor_scalar_mul(q2c_row[:], psq2c[:], Zinv[:])

                psbc = ps_s.tile([128, D], f32, tag="ps_s")
                if o["c_f32r"]:
                    nc.tensor.matmul(psbc[:], ones_row[:], q2c_row[:],
                                     start=True, stop=True)
                else:
                    nc.tensor.matmul(psbc[:], ones_row_f[:], q2c_row[:],
                                     start=True, stop=True)
                q2c_bc = constp.tile([128, D], f32, tag="q2c_bc")
                copy_op(q2c_bc[:], psbc[:])

                # ---------------- phase 3 ------------------------------------
                if o["two_pass"]:
                    for j in range(TT):
                        do_mm2_epilogue(j, q2c_bc)
                else:
                    for j in range(TT):
                        jj, jr = divmod(j, 4)
                        if o["o4_split"]:
                            mul_e4 = nc.gpsimd if j % 2 else nc.vector
                        else:
                            mul_e4 = (nc.gpsimd if o["mul_eng"] == "gpsimd"
                                      else nc.vector)
                        o4 = outp.tile([128, D], f32, tag="o4")
                        mul_e4.tensor_mul(o4[:], c_f32(c_tile(j)[:]),
                                          q2c_bc[:])
                        out_dma(j, slice(1536, 2048), o4[:])

        if timing_mode:
            with tc.tile_pool(name="tickp", bufs=1) as tickp:
                tk = tickp.tile([1, 1], f32, tag="tick")
                nc.vector.memset(tk[:], 1.0)
                nc.sync.dma_start(tick_d[:], tk[:])

    nc.compile()
    return nc


# Default kernel: the v3 s-transposed formulation (sq via the matmul path;
# tensor_tensor_reduce and Act-engine f32r writes fault on HW), with input
# DMA dispatch split across SP/Act rings, setup loads + q_bf conversion on
# the otherwise-idle Pool engine, the phase-3 o4 muls weighted toward DVE
# (Pool muls are ~2.3x slower), and deep work/out pools.
KERNEL_OPTS = {"v3": True, "sq_mm": True, "in_alt": True,
               "bufs_work": 5, "bufs_out": 6, "setup_pool": True,
               "tail_dve": True, "q_chunked": True}


def _get_built():
    global _BUILT
    if _BUILT is None:
        _BUILT = _build(opts=KERNEL_OPTS)
    return _BUILT


def kernel(c, q, w_c, b_c, w_q, b_q, w_cq, b_cq):
    """Full inputs in, full output out. Data-parallel over batch on 8 cores.

    Biases cancel mathematically (softmax shift invariance), so b_* are
    accepted but unused.
    """
    from concourse import bass_utils

    nc = _get_built()
    c = np.ascontiguousarray(np.asarray(c, dtype=np.float32))
    q = np.ascontiguousarray(np.asarray(q, dtype=np.float32))
    wc = np.ascontiguousarray(np.asarray(w_c, dtype=np.float32))
    wq = np.ascontiguousarray(np.asarray(w_q, dtype=np.float32))
    wcq = np.ascontiguousarray(np.asarray(w_cq, dtype=np.float32))

    in_maps = [
        {"c": c[b], "q": q[b], "wc": wc, "wq": wq, "wcq": wcq}
        for b in range(N_CORES)
    ]
    res = bass_utils.run_bass_kernel_spmd(
        nc, in_maps, core_ids=list(range(N_CORES)))
    return np.stack([res.results[b]["out"] for b in range(N_CORES)])

